# revision 1
# baseline (speedup 1.0000x reference)
"""Bass/Trainium2 kernel for nn_HNO_37065567764989 (self-contained).

Strategy (8 NeuronCores, SPMD):
- Branch matvec b = Wb@a column-sharded 8 ways. Each core streams its 16MB
  shard as fp16 (W scaled by 2^10 to stay normal; a carried as an fp16 hi/lo
  stationary pair, M=8 batched over 4 k-tiles per matmul). 512B AllReduce
  combines partials.
- Nx=32768 points sharded 8 ways (4096/core). Trunk layer-1 overlaps the Wb
  stream; layer-2 (z2/A/B/C, fp16 hi/lo pair stationaries+movings) fills the
  AllReduce latency; EnergyNet runs after, extracting u/u_x/u_xx/u_xxx rows
  with a c=Wt3^T b fp16 stationary and building first-layer preactivations
  from stacked hi/lo row movings (2 matmuls each via host (p,q) stationaries).
- High-sensitivity values flow as fp16 hi/lo pairs (~2^-24); low-sensitivity
  operands are single fp16. All matmuls run at 1 cy/row.
"""
import sys

for _p in ("/opt/trn_rl_repo",):
    if _p not in sys.path:
        sys.path.insert(0, _p)

import numpy as np

MP1, NX, P, HT, HE = 524288, 32768, 128, 128, 64
NCORES = 8
KSH = MP1 // NCORES        # 65536 contraction elems per core
NKT = KSH // 128           # 512 k-tiles
NCHUNK = 16
KTC = NKT // NCHUNK        # 32 k-tiles per chunk
NPTS = NX // NCORES        # 4096 points per core
FD = 512
NTRUNK = NPTS // FD        # 8 trunk tiles
NEN = NTRUNK // 2          # 4 energy tiles (two halves stacked)

_CACHE = {}


def _build():
    import concourse.bacc as bacc
    import concourse.mybir as mybir
    from concourse import tile

    f32 = mybir.dt.float32
    f16 = mybir.dt.float16
    AF = mybir.ActivationFunctionType
    ALU = mybir.AluOpType

    nc = bacc.Bacc("TRN2", target_bir_lowering=False, debug=False,
                   num_devices=NCORES)

    w_d = nc.dram_tensor("w", [NCHUNK, 128, KTC * 128], f16, kind="ExternalInput")
    a_d = nc.dram_tensor("a2", [128, NKT, 2], f16, kind="ExternalInput")
    x_d = nc.dram_tensor("x4", [4, NPTS], f16, kind="ExternalInput")
    sm = {}
    for name, shape, dt in [
        ("w11", [4, 128], f16), ("c1b", [128, 1], f32), ("bt2b", [128, 1], f32),
        ("wt2h", [128, 128], f16), ("wt2l", [128, 128], f16),
        ("w2ah", [128, 128], f16), ("w2al", [128, 128], f16),
        ("w2bh", [128, 128], f16), ("w2bl", [128, 128], f16),
        ("w2ch", [128, 128], f16), ("w2cl", [128, 128], f16),
        ("wt3h", [128, 128], f16), ("wt3l", [128, 128], f16),
        ("SEH", [12, 128], f16), ("SEL", [12, 128], f16),
        ("SPH", [12, 128], f16), ("SPL", [12, 128], f16),
        ("SPPH", [12, 128], f16), ("SPPL", [12, 128], f16),
        ("e0", [128, 128], f16), ("eq", [128, 128], f16), ("ep", [128, 128], f16),
        ("v6", [128, 6], f16),
        ("be1b2", [128, 1], f32), ("be2b2", [128, 1], f32),
        ("sel4m", [8, 4], f32),
    ]:
        sm[name] = nc.dram_tensor(name, shape, dt, kind="ExternalInput")
    out_d = nc.dram_tensor("out", [2, NPTS // 2], f32, kind="ExternalOutput")
    cc_in = nc.dram_tensor("cc_in", [128, 1], f32)
    cc_out = nc.dram_tensor("cc_out", [128, 1], f32, addr_space="Shared")

    def TT(eng, out, i0, i1, op=ALU.mult):
        eng.tensor_tensor(out, i0, i1, op)

    with tile.TileContext(nc) as tc:
        with (
            tc.tile_pool(name="smp", bufs=1) as smp,
            tc.tile_pool(name="persist", bufs=1) as persist,
            tc.tile_pool(name="wpool", bufs=3) as wpool,
            tc.tile_pool(name="scr", bufs=1) as scr,
            tc.tile_pool(name="ps8", bufs=1, space="PSUM") as ps,
        ):
            smt = {}
            for name, h in sm.items():
                t = smp.tile(list(h.shape), h.dtype, name=f"sb_{name}")
                nc.sync.dma_start(t[:], h.ap())
                smt[name] = t
            x4 = smp.tile([4, NPTS], f16, name="x4t")
            nc.sync.dma_start(x4[:], x_d.ap())
            a2 = smp.tile([128, NKT, 2], f16, name="a2t")
            nc.sync.dma_start(a2[:], a_d.ap())
            ones11 = smp.tile([1, 1], f32, name="ones11")
            nc.vector.memset(ones11[:], 1.0)

            # ---- trunk layer-1 z1 matmuls ----
            z1tags = ["pT0", "pT1", "pB", "pC"]
            z1ps = []
            for f in range(NTRUNK):
                cs = slice(f * FD, (f + 1) * FD)
                z1 = ps.tile([128, FD], f32, tag=z1tags[f % 4], name=f"z1_{f}")
                nc.tensor.matmul(z1[:], smt["w11"][:], x4[:, cs], start=True, stop=True)
                z1ps.append(z1)

            # ---- trunk layer-1 (before the matvec stream) ----
            l1 = {}
            for f in range(NTRUNK):
                z1 = z1ps[f]
                t1f = scr.tile([128, FD], f32, tag=f"t1f{f % 2}", name=f"t1f_{f}")
                nc.scalar.activation(t1f[:], z1[:], AF.Tanh, bias=smt["c1b"][:])
                t1h = persist.tile([128, FD], f16, tag=f"t1h_{f}", name=f"t1h_{f}")
                nc.scalar.copy(t1h[:], t1f[:])
                t1l = persist.tile([128, FD], f16, tag=f"t1l_{f}", name=f"t1l_{f}")
                TT(nc.vector, t1l[:], t1f[:], t1h[:], ALU.subtract)
                s1 = scr.tile([128, FD], f32, tag=f"s1_{f % 2}", name=f"s1_{f}")
                nc.scalar.square(s1[:], t1f[:])
                tp1f = scr.tile([128, FD], f32, tag=f"tp1f{f % 2}", name=f"tp1f_{f}")
                nc.vector.tensor_scalar(tp1f[:], s1[:], -1.0, 1.0, ALU.mult, ALU.add)
                tp1h = persist.tile([128, FD], f16, tag=f"tp1h_{f}", name=f"tp1h_{f}")
                nc.scalar.copy(tp1h[:], tp1f[:])
                tp1l = persist.tile([128, FD], f16, tag=f"tp1l_{f}", name=f"tp1l_{f}")
                TT(nc.vector, tp1l[:], tp1f[:], tp1h[:], ALU.subtract)
                g2m = persist.tile([128, FD], f16, tag=f"g2m_{f}", name=f"g2m_{f}")
                TT(nc.vector, g2m[:], t1f[:], tp1f[:])
                g3m = persist.tile([128, FD], f16, tag=f"g3m_{f}", name=f"g3m_{f}")
                nc.vector.scalar_tensor_tensor(
                    g3m[:], s1[:], 1.0 / 3.0, tp1f[:], ALU.subtract, ALU.mult)
                l1[f] = (t1h, t1l, tp1h, tp1l, g2m, g3m)

            # ---- trunk layer-2 wave (fills AllReduce latency) ----
            # stage-2 outputs for trunk tiles f and f+4 share one [128,1024]
            # tile (halves side by side in the free dim) so energy extracts
            # can read both halves of an energy tile from one tile family.
            sh = {}
            for f in range(NTRUNK):
                j, off = f % NEN, (f // NEN) * FD
                t1h, t1l, tp1h, tp1l, g2m, g3m = l1[f]
                if f < NEN:
                    sh[j] = tuple(
                        persist.tile([128, 2 * FD], f16, tag=f"sh{nm}_{j}",
                                     name=f"sh_{nm}_{j}")
                        for nm in ("t2h", "t2l", "P1h", "P1l", "ux2", "ux3"))
                t2h_s, t2l_s, P1h_s, P1l_s, ux2_s, ux3_s = sh[j]
                osl = slice(off, off + FD)
                z2 = ps.tile([128, FD], f32, tag="pT0", name=f"z2_{f}")
                nc.tensor.matmul(z2[:], smt["wt2h"][:], t1h[:], start=True, stop=False)
                nc.tensor.matmul(z2[:], smt["wt2h"][:], t1l[:], start=False, stop=False)
                nc.tensor.matmul(z2[:], smt["wt2l"][:], t1h[:], start=False, stop=True)
                A = ps.tile([128, FD], f32, tag="pT1", name=f"A_{f}")
                nc.tensor.matmul(A[:], smt["w2ah"][:], tp1h[:], start=True, stop=False)
                nc.tensor.matmul(A[:], smt["w2ah"][:], tp1l[:], start=False, stop=False)
                nc.tensor.matmul(A[:], smt["w2al"][:], tp1h[:], start=False, stop=True)
                B = ps.tile([128, FD], f32, tag="pB", name=f"B_{f}")
                nc.tensor.matmul(B[:], smt["w2bh"][:], g2m[:], start=True, stop=False)
                nc.tensor.matmul(B[:], smt["w2bl"][:], g2m[:], start=False, stop=True)
                C = ps.tile([128, FD], f32, tag="pC", name=f"C_{f}")
                nc.tensor.matmul(C[:], smt["w2ch"][:], g3m[:], start=True, stop=False)
                nc.tensor.matmul(C[:], smt["w2cl"][:], g3m[:], start=False, stop=True)

                t2f = scr.tile([128, FD], f32, tag="t2f", name=f"t2f_{f}")
                nc.scalar.activation(t2f[:], z2[:], AF.Tanh, bias=smt["bt2b"][:])
                nc.scalar.copy(t2h_s[:, osl], t2f[:])
                TT(nc.vector, t2l_s[:, osl], t2f[:], t2h_s[:, osl], ALU.subtract)
                s2 = scr.tile([128, FD], f32, tag="s2", name=f"s2_{f}")
                nc.scalar.square(s2[:], t2f[:])
                tp2 = scr.tile([128, FD], f32, tag="tp2", name=f"tp2_{f}")
                nc.vector.tensor_scalar(tp2[:], s2[:], -1.0, 1.0, ALU.mult, ALU.add)
                A2 = scr.tile([128, FD], f32, tag="A2", name=f"A2_{f}")
                nc.scalar.square(A2[:], A[:])
                P1f = scr.tile([128, FD], f32, tag="P1f", name=f"P1f_{f}")
                TT(nc.vector, P1f[:], tp2[:], A[:])
                nc.scalar.copy(P1h_s[:, osl], P1f[:])
                TT(nc.vector, P1l_s[:, osl], P1f[:], P1h_s[:, osl], ALU.subtract)
                M4 = scr.tile([128, FD], f32, tag="M4", name=f"M4_{f}")
                TT(nc.gpsimd, M4[:], tp2[:], A2[:])
                M5 = scr.tile([128, FD], f32, tag="M5", name=f"M5_{f}")
                TT(nc.gpsimd, M5[:], t2f[:], M4[:])
                M6 = scr.tile([128, FD], f32, tag="M6", name=f"M6_{f}")
                TT(nc.vector, M6[:], tp2[:], B[:])
                nc.vector.scalar_tensor_tensor(
                    ux2_s[:, osl], M5[:], -2.0, M6[:], ALU.mult, ALU.add)
                A3 = scr.tile([128, FD], f32, tag="A3", name=f"A3_{f}")
                TT(nc.vector, A3[:], A2[:], A[:])
                V = scr.tile([128, FD], f32, tag="V", name=f"V_{f}")
                nc.vector.scalar_tensor_tensor(
                    V[:], s2[:], 1.0 / 3.0, tp2[:], ALU.subtract, ALU.mult)
                M1 = scr.tile([128, FD], f32, tag="M1", name=f"M1_{f}")
                TT(nc.gpsimd, M1[:], V[:], A3[:])
                W1 = scr.tile([128, FD], f32, tag="W1", name=f"W1_{f}")
                TT(nc.vector, W1[:], P1f[:], B[:])
                M2 = scr.tile([128, FD], f32, tag="M2", name=f"M2_{f}")
                TT(nc.gpsimd, M2[:], t2f[:], W1[:])
                M3 = scr.tile([128, FD], f32, tag="M3", name=f"M3_{f}")
                TT(nc.vector, M3[:], tp2[:], C[:])
                D1 = scr.tile([128, FD], f32, tag="D1", name=f"D1_{f}")
                TT(nc.gpsimd, D1[:], M1[:], M2[:], ALU.subtract)
                nc.vector.scalar_tensor_tensor(
                    ux3_s[:, osl], D1[:], 6.0, M3[:], ALU.mult, ALU.add)

            # ---- matvec: stream W shard ----
            b8 = ps.tile([8, FD], f32, tag="pMV", name="b8")
            for i in range(NCHUNK):
                wch = wpool.tile([128, KTC * 128], f16, tag="wch", name="wch")
                half = KTC * 64
                nc.sync.dma_start(wch[:, 0:half], w_d.ap()[i][:, 0:half])
                nc.sync.dma_start(wch[:, half:], w_d.ap()[i][:, half:])
                for g in range(KTC // 4):
                    nc.tensor.matmul(
                        b8[:], a2[:, i * KTC + 4 * g:i * KTC + 4 * (g + 1), :],
                        wch[:, g * 512:(g + 1) * 512],
                        start=(i == 0 and g == 0),
                        stop=(i == NCHUNK - 1 and g == KTC // 4 - 1),
                    )

            # ---- local reduce + AllReduce ----
            b8sb = smp.tile([8, FD], f32, name="b8sb")
            nc.scalar.copy(b8sb[:], b8[:])
            bcol = ps.tile([128, 1], f32, tag="pBC", name="bcol")
            for j in range(4):
                nc.tensor.matmul(bcol[:], b8sb[:, j * 128:(j + 1) * 128],
                                 smt["sel4m"][:, j:j + 1],
                                 start=(j == 0), stop=(j == 3))
            b_loc = smp.tile([128, 1], f32, name="bloc")
            nc.scalar.copy(b_loc[:], bcol[:])
            nc.sync.dma_start(cc_in.ap(), b_loc[:])
            nc.gpsimd.collective_compute(
                "AllReduce", ALU.add,
                replica_groups=[list(range(NCORES))],
                ins=[cc_in.ap()], outs=[cc_out.ap()],
            )
            b_ar = smp.tile([128, 1], f32, name="bar")
            nc.sync.dma_start(b_ar[:], cc_out.ap())

            # ---- c = Wt3^T b (fp16 single stationary) ----
            b16 = smp.tile([128, 1], f16, name="b16")
            nc.scalar.copy(b16[:], b_ar[:])
            c0p = ps.tile([1, 128], f32, tag="pT0", name="c0p")
            nc.tensor.matmul(c0p[:], b16[:], smt["wt3h"][:], start=True, stop=False)
            nc.tensor.matmul(c0p[:], b16[:], smt["wt3l"][:], start=False, stop=True)
            c0 = smp.tile([1, 128], f32, name="c0")
            nc.scalar.copy(c0[:], c0p[:])
            ct = ps.tile([128, 1], f32, tag="pT1", name="ct")
            nc.tensor.matmul(ct[:], c0[:], ones11[:], start=True, stop=True)
            c16 = smp.tile([128, 1], f16, name="c16")
            nc.scalar.copy(c16[:], ct[:])

            # ---- energy phase: hoisted extracts for all tiles ----
            exttags = ["pT0", "pT1"]
            mov12s = {}
            ti = 0
            for e in range(NEN):
                t2h_s, t2l_s, P1h_s, P1l_s, ux2_s, ux3_s = sh[e]
                mov12 = scr.tile([12, FD], f16, tag=f"mv12_{e}", name=f"mv12_{e}")
                mov12s[e] = mov12
                for qi, movs in enumerate(((t2h_s, t2l_s), (P1h_s, P1l_s),
                                           (ux2_s,), (ux3_s,))):
                    hlw = 2 * FD if qi < 2 else FD
                    hlab = wpool.tile([1, 2 * hlw], f16, tag="wch",
                                      name=f"hlab{e}_{qi}")
                    for hx in range(2):
                        osl = slice(hx * FD, (hx + 1) * FD)
                        uq = ps.tile([1, FD], f32, tag=exttags[ti % 2],
                                     name=f"uq{e}_{qi}_{hx}")
                        ti += 1
                        for mi, mv in enumerate(movs):
                            nc.tensor.matmul(uq[:], c16[:], mv[:, osl],
                                             start=(mi == 0),
                                             stop=(mi == len(movs) - 1))
                        if qi < 2:
                            nc.scalar.copy(hlab[:, hx * hlw:hx * hlw + FD], uq[:])
                            TT(nc.vector, hlab[:, hx * hlw + FD:(hx + 1) * hlw],
                               uq[:], hlab[:, hx * hlw:hx * hlw + FD], ALU.subtract)
                        else:
                            nc.scalar.copy(hlab[:, hx * FD:(hx + 1) * FD], uq[:])
                    # rows: qi=0 -> 0:4 (uhA,ulA,uhB,ulB); qi=1 -> 4:8;
                    # qi=2 -> 8:10; qi=3 -> 10:12
                    r0 = qi * 4 if qi < 2 else 4 + qi * 2
                    nr = 4 if qi < 2 else 2
                    nc.sync.dma_start(mov12[r0:r0 + nr, :], hlab[:])

            for e in range(NEN):
                mov12 = mov12s[e]
                trio = [["pB", "pC", "pBC"], ["pD", "pE", "pMV"]][e % 2]
                dzt, dyt = ("pT0", "pT1") if e % 2 == 0 else ("pT1", "pT0")
                z1e = ps.tile([128, FD], f32, tag=trio[0], name=f"z1e_{e}")
                nc.tensor.matmul(z1e[:], smt["SEH"][:], mov12[:], start=True, stop=False)
                nc.tensor.matmul(z1e[:], smt["SEL"][:], mov12[:], start=False, stop=True)
                z1p = ps.tile([128, FD], f32, tag=trio[1], name=f"z1p_{e}")
                nc.tensor.matmul(z1p[:], smt["SPH"][:], mov12[:], start=True, stop=False)
                nc.tensor.matmul(z1p[:], smt["SPL"][:], mov12[:], start=False, stop=True)
                z1pp = ps.tile([128, FD], f32, tag=trio[2], name=f"z1pp_{e}")
                nc.tensor.matmul(z1pp[:], smt["SPPH"][:], mov12[:], start=True, stop=False)
                nc.tensor.matmul(z1pp[:], smt["SPPL"][:], mov12[:], start=False, stop=True)

                t1ef = scr.tile([128, FD], f32, tag="t2f", name=f"t1ef_{e}")
                nc.scalar.activation(t1ef[:], z1e[:], AF.Tanh, bias=smt["be1b2"][:])
                t1eh = scr.tile([128, FD], f16, tag="s2", name=f"t1eh_{e}")
                nc.scalar.copy(t1eh[:], t1ef[:])
                t1el = scr.tile([128, FD], f16, tag="tp2", name=f"t1el_{e}")
                TT(nc.vector, t1el[:], t1ef[:], t1eh[:], ALU.subtract)
                z1psb = scr.tile([128, FD], f16, tag="A2", name=f"z1psb_{e}")
                nc.scalar.copy(z1psb[:], z1p[:])
                z1ppsb = scr.tile([128, FD], f16, tag="P1f", name=f"z1ppsb_{e}")
                nc.scalar.copy(z1ppsb[:], z1pp[:])
                s1e = scr.tile([128, FD], f16, tag="M4", name=f"s1e_{e}")
                nc.scalar.square(s1e[:], t1ef[:])
                m_ = scr.tile([128, FD], f16, tag="M5", name=f"m_{e}")
                nc.vector.tensor_scalar(m_[:], s1e[:], -1.0, 1.0, ALU.mult, ALU.add)
                z1p2 = scr.tile([128, FD], f16, tag="M6", name=f"z1p2_{e}")
                TT(nc.gpsimd, z1p2[:], z1psb[:], z1psb[:])
                N1 = scr.tile([128, FD], f16, tag="A3", name=f"N1_{e}")
                TT(nc.gpsimd, N1[:], t1ef[:], m_[:])
                a1p = scr.tile([128, FD], f16, tag="V", name=f"a1p_{e}")
                TT(nc.vector, a1p[:], m_[:], z1psb[:])
                N2 = scr.tile([128, FD], f16, tag="M1", name=f"N2_{e}")
                TT(nc.gpsimd, N2[:], N1[:], z1p2[:])
                N3 = scr.tile([128, FD], f16, tag="W1", name=f"N3_{e}")
                TT(nc.vector, N3[:], m_[:], z1ppsb[:])
                zin = scr.tile([128, FD], f16, tag="M2", name=f"zin_{e}")
                nc.vector.scalar_tensor_tensor(
                    zin[:], N2[:], -2.0, N3[:], ALU.mult, ALU.add)
                mpc = scr.tile([128, FD], f16, tag="M3", name=f"mpc_{e}")
                TT(nc.vector, mpc[:], N1[:], z1psb[:])
                O1 = scr.tile([128, FD], f16, tag="D1", name=f"O1_{e}")
                nc.vector.scalar_tensor_tensor(
                    O1[:], s1e[:], 1.0 / 3.0, m_[:], ALU.subtract, ALU.mult)
                O2f = scr.tile([128, FD], f16, tag="t1f0", name=f"O2f_{e}")
                TT(nc.gpsimd, O2f[:], O1[:], z1p2[:])
                O3f = scr.tile([128, FD], f16, tag="t1f1", name=f"O3f_{e}")
                TT(nc.vector, O3f[:], N1[:], z1ppsb[:])
                O2m = scr.tile([128, FD], f16, tag="s1_0", name=f"O2m_{e}")
                nc.vector.scalar_tensor_tensor(
                    O2m[:], O2f[:], 3.0, O3f[:], ALU.mult, ALU.subtract)

                z2e = ps.tile([128, FD], f32, tag=trio[0], name=f"z2e_{e}")
                nc.tensor.matmul(z2e[:], smt["e0"][:], t1eh[:], start=True, stop=False)
                nc.tensor.matmul(z2e[:], smt["e0"][:], t1el[:], start=False, stop=True)
                z2ep = ps.tile([128, FD], f32, tag=trio[1], name=f"z2ep_{e}")
                nc.tensor.matmul(z2ep[:], smt["e0"][:], a1p[:], start=True, stop=True)
                z2epp = ps.tile([128, FD], f32, tag=trio[2], name=f"z2epp_{e}")
                nc.tensor.matmul(z2epp[:], smt["e0"][:], zin[:], start=True, stop=True)
                Dz = ps.tile([128, FD], f32, tag=dzt, name=f"Dz_{e}")
                nc.tensor.matmul(Dz[:], smt["eq"][:], m_[:], start=True, stop=True)
                DyN = ps.tile([128, FD], f32, tag=dyt, name=f"DyN_{e}")
                nc.tensor.matmul(DyN[:], smt["ep"][:], m_[:], start=True, stop=True)
                DzpN = ps.tile([128, FD], f32, tag=trio[0], name=f"DzpN_{e}")
                nc.tensor.matmul(DzpN[:], smt["eq"][:], mpc[:], start=True, stop=True)
                DypN = ps.tile([128, FD], f32, tag=trio[1], name=f"DypN_{e}")
                nc.tensor.matmul(DypN[:], smt["ep"][:], mpc[:], start=True, stop=True)
                Dzpp2 = ps.tile([128, FD], f32, tag=trio[2], name=f"Dzpp2_{e}")
                nc.tensor.matmul(Dzpp2[:], smt["eq"][:], O2m[:], start=True, stop=True)

                t2e = scr.tile([128, FD], f16, tag="s1_1", name=f"t2e_{e}")
                nc.scalar.activation(t2e[:], z2e[:], AF.Tanh, bias=smt["be2b2"][:])
                s2e = scr.tile([128, FD], f16, tag="tp1f0", name=f"s2e_{e}")
                TT(nc.vector, s2e[:], t2e[:], t2e[:])
                w_ = scr.tile([128, FD], f16, tag="tp1f1", name=f"w_{e}")
                nc.vector.tensor_scalar(w_[:], s2e[:], -1.0, 1.0, ALU.mult, ALU.add)
                z2ep16 = scr.tile([128, FD], f16, tag="z2ep16", name=f"z2ep16_{e}")
                nc.scalar.copy(z2ep16[:], z2ep[:])
                z2ep2 = scr.tile([128, FD], f16, tag="z2ep2", name=f"z2ep2_{e}")
                TT(nc.gpsimd, z2ep2[:], z2ep16[:], z2ep16[:])
                Q1 = scr.tile([128, FD], f16, tag="Q1", name=f"Q1_{e}")
                TT(nc.gpsimd, Q1[:], t2e[:], w_[:])
                wpc = scr.tile([128, FD], f16, tag="wpc", name=f"wpc_{e}")
                TT(nc.vector, wpc[:], Q1[:], z2ep16[:])
                R1 = scr.tile([128, FD], f16, tag="R1", name=f"R1_{e}")
                nc.vector.scalar_tensor_tensor(
                    R1[:], s2e[:], 1.0 / 3.0, w_[:], ALU.subtract, ALU.mult)
                R2f = scr.tile([128, FD], f16, tag="R2f", name=f"R2f_{e}")
                TT(nc.gpsimd, R2f[:], R1[:], z2ep2[:])
                R3f = scr.tile([128, FD], f16, tag="R3f", name=f"R3f_{e}")
                TT(nc.vector, R3f[:], Q1[:], z2epp[:])
                t1m = scr.tile([128, FD], f16, tag="t1m", name=f"t1m_{e}")
                nc.vector.scalar_tensor_tensor(
                    t1m[:], R2f[:], 3.0, R3f[:], ALU.mult, ALU.subtract)
                F1 = scr.tile([128, FD], f16, tag="F1", name=f"F1_{e}")
                TT(nc.vector, F1[:], t1m[:], Dz[:])
                DyNs = scr.tile([128, FD], f16, tag="DyNs", name=f"DyNs_{e}")
                nc.scalar.copy(DyNs[:], DyN[:])
                t2m = scr.tile([128, FD], f16, tag="t2m", name=f"t2m_{e}")
                nc.vector.scalar_tensor_tensor(
                    t2m[:], DzpN[:], 4.0, DyNs[:], ALU.mult, ALU.add)
                F2 = scr.tile([128, FD], f16, tag="F2", name=f"F2_{e}")
                TT(nc.gpsimd, F2[:], wpc[:], t2m[:])
                DypNs = scr.tile([128, FD], f16, tag="DypNs", name=f"DypNs_{e}")
                nc.scalar.copy(DypNs[:], DypN[:])
                t3m = scr.tile([128, FD], f16, tag="t3m", name=f"t3m_{e}")
                TT(nc.vector, t3m[:], Dzpp2[:], DypNs[:], ALU.add)
                F3 = scr.tile([128, FD], f16, tag="F3", name=f"F3_{e}")
                TT(nc.vector, F3[:], w_[:], t3m[:])

                vps = ps.tile([2, FD], f32, tag=trio[1], name=f"vps_{e}")
                nc.tensor.matmul(vps[:], smt["v6"][:, 0:2], F1[:], start=True, stop=False)
                nc.tensor.matmul(vps[:], smt["v6"][:, 2:4], F2[:], start=False, stop=False)
                nc.tensor.matmul(vps[:], smt["v6"][:, 4:6], F3[:], start=False, stop=True)
                ot = scr.tile([2, FD], f32, tag="ot", name=f"ot_{e}")
                nc.scalar.copy(ot[:], vps[:])
                nc.sync.dma_start(out_d.ap()[:, e * FD:(e + 1) * FD], ot[:])

    nc.compile()
    return nc


def _get_nc():
    if "nc" not in _CACHE:
        _CACHE["nc"] = _build()
    return _CACHE["nc"]


def kernel(**inputs):
    import concourse.bass_utils as bass_utils

    f = lambda k: np.asarray(inputs[k], np.float32)
    a, x, t = f("a"), f("x"), np.float32(inputs["t"])
    Wb, Wt1, bt1, Wt2, bt2 = f("Wb"), f("Wt1"), f("bt1"), f("Wt2"), f("bt2")
    Wt3, We1, be1, We2, be2, We3 = (
        f("Wt3"), f("We1"), f("be1"), f("We2"), f("be2"), f("We3"))

    h16 = lambda v: np.asarray(v, np.float32).astype(np.float16)
    def pair16(v):
        h = h16(v)
        return h, h16(np.asarray(v, np.float32) - h.astype(np.float32))

    w1 = Wt1[:, 0]
    c1b = (Wt1[:, 1] * t + bt1)[:, None]
    w1h, w1l = pair16(w1)
    w11 = np.stack([w1h, w1h, w1l, w1l])                       # [4,128]
    wt2t = np.ascontiguousarray(Wt2.T)
    mk = lambda M: pair16(M)
    wt2h, wt2l = mk(wt2t)
    w2ah, w2al = mk(wt2t * w1[:, None])
    w2bh, w2bl = mk(wt2t * (-2.0 * w1 ** 2)[:, None])
    w2ch, w2cl = mk(wt2t * (6.0 * w1 ** 3)[:, None])
    wt3h, wt3l = mk(Wt3)

    p, q, v = We1[:, 0], We1[:, 1], We3[0]
    ph, pl = pair16(p)
    qh, ql = pair16(q)
    # mov12 rows: 0 uhA, 1 ulA, 2 uhB, 3 ulB, 4 uxhA, 5 uxlA, 6 uxhB, 7 uxlB,
    #             8 uxxA, 9 uxxB, 10 uxxxA, 11 uxxxB
    A_, B_ = slice(0, 64), slice(64, 128)
    def stat12(rows):
        S = np.zeros((12, 128), np.float16)
        for r, vec, cs in rows:
            S[r, cs] = vec
        return S
    SEH = stat12([(0, ph, A_), (1, ph, A_), (2, ph, B_), (3, ph, B_),
                  (4, qh, A_), (5, qh, A_), (6, qh, B_), (7, qh, B_)])
    SEL = stat12([(0, pl, A_), (2, pl, B_), (4, ql, A_), (6, ql, B_)])
    SPH = stat12([(4, ph, A_), (5, ph, A_), (6, ph, B_), (7, ph, B_),
                  (8, qh, A_), (9, qh, B_)])
    SPL = stat12([(4, pl, A_), (6, pl, B_), (8, ql, A_), (9, ql, B_)])
    SPPH = stat12([(8, ph, A_), (9, ph, B_), (10, qh, A_), (11, qh, B_)])
    SPPL = stat12([(8, pl, A_), (9, pl, B_), (10, ql, A_), (11, ql, B_)])

    blk = lambda M: np.block([[M, np.zeros_like(M)], [np.zeros_like(M), M]])
    We2T = We2.T
    e0 = h16(blk(We2T))
    eq = h16(blk(We2T * q[:, None]))
    ep = h16(blk(We2T * p[:, None]))
    v6 = np.zeros((128, 6), np.float16)
    for i in range(3):
        v6[0:64, 2 * i] = h16(2.0 * v)
        v6[64:128, 2 * i + 1] = h16(2.0 * v)
    sel4m = np.zeros((8, 4), np.float32)
    for j in range(4):
        sel4m[2 * j, j] = 1.0
        sel4m[2 * j + 1, j] = 1.0

    smalls = {
        "w11": w11, "c1b": c1b.astype(np.float32), "bt2b": bt2[:, None].astype(np.float32),
        "wt2h": wt2h, "wt2l": wt2l, "w2ah": w2ah, "w2al": w2al,
        "w2bh": w2bh, "w2bl": w2bl, "w2ch": w2ch, "w2cl": w2cl,
        "wt3h": wt3h, "wt3l": wt3l,
        "SEH": SEH, "SEL": SEL, "SPH": SPH, "SPL": SPL, "SPPH": SPPH, "SPPL": SPPL,
        "e0": e0, "eq": eq, "ep": ep, "v6": v6,
        "be1b2": np.concatenate([be1, be1])[:, None].astype(np.float32),
        "be2b2": np.concatenate([be2, be2])[:, None].astype(np.float32),
        "sel4m": sel4m,
    }
    smalls = {k: np.ascontiguousarray(val) for k, val in smalls.items()}

    in_maps = []
    for c in range(NCORES):
        blk_w = Wb[:, c * KSH:(c + 1) * KSH]                   # [128, 65536]
        tr = blk_w.T.reshape(NKT, 128, 128).transpose(1, 0, 2)  # [k1, kt, p]
        tr = tr.reshape(128, NCHUNK, KTC * 128).transpose(1, 0, 2)
        wsh = np.ascontiguousarray(h16(1024.0 * tr))           # [16,128,4096]
        ash = (a[c * KSH:(c + 1) * KSH] / 1024.0).reshape(NKT, 128).T  # [k1, kt]
        ah, al = pair16(ash)
        a2 = np.ascontiguousarray(np.stack([ah, al], axis=2))  # [128,512,2]
        xs = x[c * NPTS:(c + 1) * NPTS]
        xh, xl = pair16(xs)
        x4 = np.ascontiguousarray(np.stack([xh, xl, xh, xl]))  # [4,4096]
        im = {"w": wsh, "a2": a2, "x4": x4}
        im.update(smalls)
        in_maps.append(im)

    global _last_in_maps
    _last_in_maps = in_maps
    nc = _get_nc()
    res = bass_utils.run_bass_kernel_spmd(nc, in_maps, core_ids=list(range(NCORES)))
    outs = []
    for c in range(NCORES):
        o = res.results[c]["out"]          # [2, NPTS//2]
        outs.append(np.asarray(o).reshape(-1))
    return np.concatenate(outs).astype(np.float32)



# revision 3
# speedup vs baseline: 1.1851x; 1.1851x over previous
"""Bass/Trainium2 kernel for nn_HNO_37065567764989 (self-contained).

Strategy (8 NeuronCores, SPMD):
- Branch matvec b = Wb@a column-sharded 8 ways. Each core streams its 16MB
  shard as fp16 (W scaled by 2^10; a as an fp16 hi/lo stationary pair), one
  1MB DMA per chunk for full SDMA fan-out. 512B AllReduce combines partials.
- Nx=32768 points sharded 8 ways (4096/core). Trunk layer-1 + layer-2 wave
  overlap the stream. GpSimd carries only early-tile (f<4) products, the
  local-reduce copies and the collective, so the mesh wait never blocks the
  trunk tail (f>=4 products go to vector). The b->c chain runs on gpsimd
  right after the mesh.
- Precision: t2/P1/u/u_x flow as fp16 hi/lo pairs; t1/tp1 and the B/C/
  EnergyNet first-layer stationaries are single fp16 (validated 1.26e-2).
"""
import sys

for _p in ("/opt/trn_rl_repo",):
    if _p not in sys.path:
        sys.path.insert(0, _p)

import numpy as np

MP1, NX, P, HT, HE = 524288, 32768, 128, 128, 64
NCORES = 8
KSH = MP1 // NCORES        # 65536 contraction elems per core
NKT = KSH // 128           # 512 k-tiles
NCHUNK = 16
KTC = NKT // NCHUNK        # 32 k-tiles per chunk
NPTS = NX // NCORES        # 4096 points per core
FD = 512
NTRUNK = NPTS // FD        # 8 trunk tiles
NEN = NTRUNK // 2          # 4 energy tiles (two halves stacked)

# packed-constant column offsets (fp16 pack B: trunk layer-2 stationaries)
_PKB = {"wt2h": 0, "wt2l": 128, "w2ah": 256, "w2al": 384, "w2b": 512,
        "w2c": 640}
PKB_COLS = 768
# fp16 pack C: branch/energy stationaries
_PKC = {"wt3h": 0, "wt3l": 128, "e0": 256, "eq": 384, "ep": 512,
        "SEH": 640, "SPH": 768, "SPPH": 896, "v6": 1024}
PKC_COLS = 1030
# f32 pack: per-column biases + reduce selector
_PK32 = {"c1b": 0, "bt2b": 1, "be1b2": 2, "be2b2": 3, "sel4m": 4}
PK32_COLS = 8

_CACHE = {}


def _build():
    import concourse.bacc as bacc
    import concourse.mybir as mybir
    from concourse import tile

    f32 = mybir.dt.float32
    f16 = mybir.dt.float16
    AF = mybir.ActivationFunctionType
    ALU = mybir.AluOpType

    nc = bacc.Bacc("TRN2", target_bir_lowering=False, debug=False,
                   num_devices=NCORES)

    w_d = nc.dram_tensor("w", [NCHUNK, 128, KTC * 128], f16, kind="ExternalInput")
    a_d = nc.dram_tensor("a2", [128, NKT, 2], f16, kind="ExternalInput")
    x_d = nc.dram_tensor("x4", [4, NPTS], f16, kind="ExternalInput")
    w11_d = nc.dram_tensor("w11", [4, 128], f16, kind="ExternalInput")
    pkb_d = nc.dram_tensor("pkb", [128, PKB_COLS], f16, kind="ExternalInput")
    pkc_d = nc.dram_tensor("pkc", [128, PKC_COLS], f16, kind="ExternalInput")
    pk32_d = nc.dram_tensor("pk32", [128, PK32_COLS], f32, kind="ExternalInput")
    out_d = nc.dram_tensor("out", [2, NPTS // 2], f32, kind="ExternalOutput")
    cc_in = nc.dram_tensor("cc_in", [128, 1], f32)
    cc_out = nc.dram_tensor("cc_out", [128, 1], f32, addr_space="Shared")

    def TT(eng, out, i0, i1, op=ALU.mult):
        eng.tensor_tensor(out, i0, i1, op)

    with tile.TileContext(nc) as tc:
        with (
            tc.tile_pool(name="smp", bufs=1) as smp,
            tc.tile_pool(name="persist", bufs=1) as persist,
            tc.tile_pool(name="wpool", bufs=3) as wpool,
            tc.tile_pool(name="scr", bufs=1) as scr,
            tc.tile_pool(name="ps8", bufs=1, space="PSUM") as ps,
        ):
            # ---- packed constant loads (6 DMA issues) ----
            x4 = smp.tile([4, NPTS], f16, name="x4t")
            nc.sync.dma_start(x4[:], x_d.ap())
            w11 = smp.tile([4, 128], f16, name="w11t")
            nc.sync.dma_start(w11[:], w11_d.ap())
            pk32 = smp.tile([128, PK32_COLS], f32, name="pk32t")
            nc.sync.dma_start(pk32[:], pk32_d.ap())
            a2 = smp.tile([128, NKT, 2], f16, name="a2t")
            nc.sync.dma_start(a2[:], a_d.ap())
            pkb = smp.tile([128, PKB_COLS], f16, name="pkbt")
            nc.sync.dma_start(pkb[:], pkb_d.ap())
            pkc = smp.tile([128, PKC_COLS], f16, name="pkct")
            nc.sync.dma_start(pkc[:], pkc_d.ap())

            smt = {"w11": w11[:]}
            for n_, c0 in _PKB.items():
                smt[n_] = pkb[:, c0:c0 + 128]
            for n_, c0 in _PKC.items():
                w = 6 if n_ == "v6" else 128
                r = 12 if n_ in ("SEH", "SPH", "SPPH") else 128
                smt[n_] = pkc[0:r, c0:c0 + w]
            for n_, c0 in _PK32.items():
                if n_ == "sel4m":
                    smt[n_] = pk32[0:8, c0:c0 + 4]
                else:
                    smt[n_] = pk32[:, c0:c0 + 1]
            ones11 = smp.tile([1, 1], f32, name="ones11")
            nc.vector.memset(ones11[:], 1.0)

            # ---- trunk layer-1 z1 matmuls ----
            z1tags = ["pT0", "pT1", "pB", "pC"]
            z1ps = []
            for f in range(NTRUNK):
                cs = slice(f * FD, (f + 1) * FD)
                z1 = ps.tile([128, FD], f32, tag=z1tags[f % 4], name=f"z1_{f}")
                nc.tensor.matmul(z1[:], smt["w11"], x4[:, cs], start=True, stop=True)
                z1ps.append(z1)

            # ---- trunk layer-1 elementwise (single-fp16 t1/tp1) ----
            l1 = {}
            for f in range(NTRUNK):
                z1 = z1ps[f]
                t1f = scr.tile([128, FD], f32, tag=f"t1f{f % 2}", name=f"t1f_{f}")
                nc.scalar.activation(t1f[:], z1[:], AF.Tanh, bias=smt["c1b"])
                t1h = persist.tile([128, FD], f16, tag=f"t1h_{f}", name=f"t1h_{f}")
                nc.scalar.copy(t1h[:], t1f[:])
                s1 = scr.tile([128, FD], f32, tag=f"s1_{f % 2}", name=f"s1_{f}")
                nc.scalar.square(s1[:], t1f[:])
                tp1f = scr.tile([128, FD], f32, tag=f"tp1f{f % 2}", name=f"tp1f_{f}")
                nc.vector.tensor_scalar(tp1f[:], s1[:], -1.0, 1.0, ALU.mult, ALU.add)
                tp1h = persist.tile([128, FD], f16, tag=f"tp1h_{f}", name=f"tp1h_{f}")
                nc.scalar.copy(tp1h[:], tp1f[:])
                g2m = persist.tile([128, FD], f16, tag=f"g2m_{f}", name=f"g2m_{f}")
                TT(nc.vector, g2m[:], t1f[:], tp1f[:])
                g3m = persist.tile([128, FD], f16, tag=f"g3m_{f}", name=f"g3m_{f}")
                nc.vector.scalar_tensor_tensor(
                    g3m[:], s1[:], 1.0 / 3.0, tp1f[:], ALU.subtract, ALU.mult)
                l1[f] = (t1h, tp1h, g2m, g3m)

            # ---- matvec: stream W shard (1MB DMA per chunk) ----
            b8 = ps.tile([8, FD], f32, tag="pMV", name="b8")
            for i in range(NCHUNK):
                wch = wpool.tile([128, KTC * 128], f16, tag="wch", name="wch")
                nc.sync.dma_start(wch[:], w_d.ap()[i])
                for g in range(KTC // 4):
                    nc.tensor.matmul(
                        b8[:], a2[:, i * KTC + 4 * g:i * KTC + 4 * (g + 1), :],
                        wch[:, g * 512:(g + 1) * 512],
                        start=(i == 0 and g == 0),
                        stop=(i == NCHUNK - 1 and g == KTC // 4 - 1),
                    )

            # ---- local reduce (gpsimd copies) + AllReduce ----
            b8sb = smp.tile([8, FD], f32, name="b8sb")
            nc.scalar.copy(b8sb[:], b8[:])
            bcol = ps.tile([128, 1], f32, tag="pBC", name="bcol")
            for j in range(4):
                nc.tensor.matmul(bcol[:], b8sb[:, j * 128:(j + 1) * 128],
                                 smt["sel4m"][:, j:j + 1],
                                 start=(j == 0), stop=(j == 3))
            b_loc = smp.tile([128, 1], f32, name="bloc")
            nc.scalar.copy(b_loc[:], bcol[:])
            nc.sync.dma_start(cc_in.ap(), b_loc[:])
            nc.gpsimd.collective_compute(
                "AllReduce", ALU.add,
                replica_groups=[list(range(NCORES))],
                ins=[cc_in.ap()], outs=[cc_out.ap()],
            )
            b_ar = smp.tile([128, 1], f32, name="bar")
            nc.sync.dma_start(b_ar[:], cc_out.ap())

            # ---- c = Wt3^T b (gpsimd copies: run right after the mesh) ----
            b16 = smp.tile([128, 1], f16, name="b16")
            nc.scalar.copy(b16[:], b_ar[:])
            c0p = ps.tile([1, 128], f32, tag="pBC", name="c0p")
            nc.tensor.matmul(c0p[:], b16[:], smt["wt3h"], start=True, stop=False)
            nc.tensor.matmul(c0p[:], b16[:], smt["wt3l"], start=False, stop=True)
            c0 = smp.tile([1, 128], f32, name="c0")
            nc.scalar.copy(c0[:], c0p[:])
            ct = ps.tile([128, 1], f32, tag="pBC", name="ct")
            nc.tensor.matmul(ct[:], c0[:], ones11[:], start=True, stop=True)
            c16 = smp.tile([128, 1], f16, name="c16")
            nc.scalar.copy(c16[:], ct[:])

            # ---- trunk layer-2 wave ----
            # stage-2 outputs for trunk tiles f and f+4 share one [128,1024]
            # tile (halves side by side) so energy extracts read both halves
            # of an energy tile from one tile family.
            sh = {}
            for f in range(NTRUNK):
                j, off = f % NEN, (f // NEN) * FD
                t1h, tp1h, g2m, g3m = l1[f]
                if f < NEN:
                    sh[j] = tuple(
                        persist.tile([128, 2 * FD], f16, tag=f"sh{nm}_{j}",
                                     name=f"sh_{nm}_{j}")
                        for nm in ("t2h", "t2l", "P1h", "P1l", "ux2", "ux3"))
                t2h_s, t2l_s, P1h_s, P1l_s, ux2_s, ux3_s = sh[j]
                osl = slice(off, off + FD)
                # gpsimd only for early tiles: keeps the mesh off the
                # critical path of the trunk tail
                ge = nc.gpsimd if f < 4 else nc.vector
                zt, at = ("pT0", "pT1") if f % 2 == 0 else ("pT2", "pT3")
                z2 = ps.tile([128, FD], f32, tag=zt, name=f"z2_{f}")
                nc.tensor.matmul(z2[:], smt["wt2h"], t1h[:], start=True, stop=False)
                nc.tensor.matmul(z2[:], smt["wt2l"], t1h[:], start=False, stop=True)
                A = ps.tile([128, FD], f32, tag=at, name=f"A_{f}")
                nc.tensor.matmul(A[:], smt["w2ah"], tp1h[:], start=True, stop=False)
                nc.tensor.matmul(A[:], smt["w2al"], tp1h[:], start=False, stop=True)
                B = ps.tile([128, FD], f32, tag="pB", name=f"B_{f}")
                nc.tensor.matmul(B[:], smt["w2b"], g2m[:], start=True, stop=True)
                C = ps.tile([128, FD], f32, tag="pC", name=f"C_{f}")
                nc.tensor.matmul(C[:], smt["w2c"], g3m[:], start=True, stop=True)

                t2f = scr.tile([128, FD], f32, tag="t2f", name=f"t2f_{f}")
                nc.scalar.activation(t2f[:], z2[:], AF.Tanh, bias=smt["bt2b"])
                nc.scalar.copy(t2h_s[:, osl], t2f[:])
                TT(nc.vector, t2l_s[:, osl], t2f[:], t2h_s[:, osl], ALU.subtract)
                s2 = scr.tile([128, FD], f32, tag="s2", name=f"s2_{f}")
                nc.scalar.square(s2[:], t2f[:])
                tp2 = scr.tile([128, FD], f32, tag="tp2", name=f"tp2_{f}")
                nc.vector.tensor_scalar(tp2[:], s2[:], -1.0, 1.0, ALU.mult, ALU.add)
                A2 = scr.tile([128, FD], f32, tag="A2", name=f"A2_{f}")
                nc.scalar.square(A2[:], A[:])
                P1f = scr.tile([128, FD], f32, tag="P1f", name=f"P1f_{f}")
                TT(nc.vector, P1f[:], tp2[:], A[:])
                nc.scalar.copy(P1h_s[:, osl], P1f[:])
                TT(nc.vector, P1l_s[:, osl], P1f[:], P1h_s[:, osl], ALU.subtract)
                M4 = scr.tile([128, FD], f32, tag="M4", name=f"M4_{f}")
                TT(ge, M4[:], tp2[:], A2[:])
                M5 = scr.tile([128, FD], f32, tag="M5", name=f"M5_{f}")
                TT(ge, M5[:], t2f[:], M4[:])
                M6 = scr.tile([128, FD], f32, tag="M6", name=f"M6_{f}")
                TT(nc.vector, M6[:], tp2[:], B[:])
                nc.vector.scalar_tensor_tensor(
                    ux2_s[:, osl], M5[:], -2.0, M6[:], ALU.mult, ALU.add)
                A3 = scr.tile([128, FD], f32, tag="A3", name=f"A3_{f}")
                TT(nc.vector, A3[:], A2[:], A[:])
                V = scr.tile([128, FD], f32, tag="V", name=f"V_{f}")
                nc.vector.scalar_tensor_tensor(
                    V[:], s2[:], 1.0 / 3.0, tp2[:], ALU.subtract, ALU.mult)
                M1 = scr.tile([128, FD], f32, tag="M1", name=f"M1_{f}")
                TT(ge, M1[:], V[:], A3[:])
                W1 = scr.tile([128, FD], f32, tag="W1", name=f"W1_{f}")
                TT(nc.vector, W1[:], P1f[:], B[:])
                M2 = scr.tile([128, FD], f32, tag="M2", name=f"M2_{f}")
                TT(ge, M2[:], t2f[:], W1[:])
                M3 = scr.tile([128, FD], f32, tag="M3", name=f"M3_{f}")
                TT(nc.vector, M3[:], tp2[:], C[:])
                D1 = scr.tile([128, FD], f32, tag="D1", name=f"D1_{f}")
                TT(ge, D1[:], M1[:], M2[:], ALU.subtract)
                nc.vector.scalar_tensor_tensor(
                    ux3_s[:, osl], D1[:], 6.0, M3[:], ALU.mult, ALU.add)

            # ---- energy phase: hoisted extracts for all tiles ----
            exttags = ["pT0", "pT1"]
            mov12s = {}
            ti = 0
            for e in range(NEN):
                t2h_s, t2l_s, P1h_s, P1l_s, ux2_s, ux3_s = sh[e]
                mov12 = scr.tile([12, FD], f16, tag=f"mv12_{e}", name=f"mv12_{e}")
                mov12s[e] = mov12
                for qi, movs in enumerate(((t2h_s, t2l_s), (P1h_s, P1l_s),
                                           (ux2_s,), (ux3_s,))):
                    hlw = 2 * FD if qi < 2 else FD
                    hlab = wpool.tile([1, 2 * hlw], f16, tag="wch",
                                      name=f"hlab{e}_{qi}")
                    for hx in range(2):
                        osl = slice(hx * FD, (hx + 1) * FD)
                        uq = ps.tile([1, FD], f32, tag=exttags[ti % 2],
                                     name=f"uq{e}_{qi}_{hx}")
                        ti += 1
                        for mi, mv in enumerate(movs):
                            nc.tensor.matmul(uq[:], c16[:], mv[:, osl],
                                             start=(mi == 0),
                                             stop=(mi == len(movs) - 1))
                        if qi < 2:
                            nc.scalar.copy(hlab[:, hx * hlw:hx * hlw + FD], uq[:])
                            TT(nc.vector, hlab[:, hx * hlw + FD:(hx + 1) * hlw],
                               uq[:], hlab[:, hx * hlw:hx * hlw + FD], ALU.subtract)
                        else:
                            nc.scalar.copy(hlab[:, hx * FD:(hx + 1) * FD], uq[:])
                    # rows: qi=0 -> 0:4 (uhA,ulA,uhB,ulB); qi=1 -> 4:8;
                    # qi=2 -> 8:10; qi=3 -> 10:12
                    r0 = qi * 4 if qi < 2 else 4 + qi * 2
                    nr = 4 if qi < 2 else 2
                    nc.sync.dma_start(mov12[r0:r0 + nr, :], hlab[:])

            for e in range(NEN):
                mov12 = mov12s[e]
                trio = [["pB", "pC", "pBC"], ["pT2", "pT3", "pMV"]][e % 2]
                dzt, dyt = ("pT0", "pT1") if e % 2 == 0 else ("pT1", "pT0")
                z1e = ps.tile([128, FD], f32, tag=trio[0], name=f"z1e_{e}")
                nc.tensor.matmul(z1e[:], smt["SEH"], mov12[:], start=True, stop=True)
                z1p = ps.tile([128, FD], f32, tag=trio[1], name=f"z1p_{e}")
                nc.tensor.matmul(z1p[:], smt["SPH"], mov12[:], start=True, stop=True)
                z1pp = ps.tile([128, FD], f32, tag=trio[2], name=f"z1pp_{e}")
                nc.tensor.matmul(z1pp[:], smt["SPPH"], mov12[:], start=True, stop=True)

                t1ef = scr.tile([128, FD], f32, tag="t2f", name=f"t1ef_{e}")
                nc.scalar.activation(t1ef[:], z1e[:], AF.Tanh, bias=smt["be1b2"])
                t1eh = scr.tile([128, FD], f16, tag="s2", name=f"t1eh_{e}")
                nc.scalar.copy(t1eh[:], t1ef[:])
                z1psb = scr.tile([128, FD], f16, tag="A2", name=f"z1psb_{e}")
                nc.scalar.copy(z1psb[:], z1p[:])
                z1ppsb = scr.tile([128, FD], f16, tag="P1f", name=f"z1ppsb_{e}")
                nc.scalar.copy(z1ppsb[:], z1pp[:])
                s1e = scr.tile([128, FD], f16, tag="M4", name=f"s1e_{e}")
                nc.scalar.square(s1e[:], t1ef[:])
                m_ = scr.tile([128, FD], f16, tag="M5", name=f"m_{e}")
                nc.vector.tensor_scalar(m_[:], s1e[:], -1.0, 1.0, ALU.mult, ALU.add)
                z1p2 = scr.tile([128, FD], f16, tag="M6", name=f"z1p2_{e}")
                TT(nc.gpsimd, z1p2[:], z1psb[:], z1psb[:])
                N1 = scr.tile([128, FD], f16, tag="A3", name=f"N1_{e}")
                TT(nc.gpsimd, N1[:], t1ef[:], m_[:])
                a1p = scr.tile([128, FD], f16, tag="V", name=f"a1p_{e}")
                TT(nc.vector, a1p[:], m_[:], z1psb[:])
                N2 = scr.tile([128, FD], f16, tag="M1", name=f"N2_{e}")
                TT(nc.gpsimd, N2[:], N1[:], z1p2[:])
                N3 = scr.tile([128, FD], f16, tag="W1", name=f"N3_{e}")
                TT(nc.vector, N3[:], m_[:], z1ppsb[:])
                zin = scr.tile([128, FD], f16, tag="M2", name=f"zin_{e}")
                nc.vector.scalar_tensor_tensor(
                    zin[:], N2[:], -2.0, N3[:], ALU.mult, ALU.add)
                mpc = scr.tile([128, FD], f16, tag="M3", name=f"mpc_{e}")
                TT(nc.vector, mpc[:], N1[:], z1psb[:])
                O1 = scr.tile([128, FD], f16, tag="D1", name=f"O1_{e}")
                nc.vector.scalar_tensor_tensor(
                    O1[:], s1e[:], 1.0 / 3.0, m_[:], ALU.subtract, ALU.mult)
                O2f = scr.tile([128, FD], f16, tag="t1f0", name=f"O2f_{e}")
                TT(nc.gpsimd, O2f[:], O1[:], z1p2[:])
                O3f = scr.tile([128, FD], f16, tag="t1f1", name=f"O3f_{e}")
                TT(nc.vector, O3f[:], N1[:], z1ppsb[:])
                O2m = scr.tile([128, FD], f16, tag="s1_0", name=f"O2m_{e}")
                nc.vector.scalar_tensor_tensor(
                    O2m[:], O2f[:], 3.0, O3f[:], ALU.mult, ALU.subtract)

                z2e = ps.tile([128, FD], f32, tag=trio[0], name=f"z2e_{e}")
                nc.tensor.matmul(z2e[:], smt["e0"], t1eh[:], start=True, stop=True)
                z2ep = ps.tile([128, FD], f32, tag=trio[1], name=f"z2ep_{e}")
                nc.tensor.matmul(z2ep[:], smt["e0"], a1p[:], start=True, stop=True)
                z2epp = ps.tile([128, FD], f32, tag=trio[2], name=f"z2epp_{e}")
                nc.tensor.matmul(z2epp[:], smt["e0"], zin[:], start=True, stop=True)
                Dz = ps.tile([128, FD], f32, tag=dzt, name=f"Dz_{e}")
                nc.tensor.matmul(Dz[:], smt["eq"], m_[:], start=True, stop=True)
                DyN = ps.tile([128, FD], f32, tag=dyt, name=f"DyN_{e}")
                nc.tensor.matmul(DyN[:], smt["ep"], m_[:], start=True, stop=True)
                DzpN = ps.tile([128, FD], f32, tag=trio[0], name=f"DzpN_{e}")
                nc.tensor.matmul(DzpN[:], smt["eq"], mpc[:], start=True, stop=True)
                DypN = ps.tile([128, FD], f32, tag=trio[1], name=f"DypN_{e}")
                nc.tensor.matmul(DypN[:], smt["ep"], mpc[:], start=True, stop=True)
                Dzpp2 = ps.tile([128, FD], f32, tag=trio[2], name=f"Dzpp2_{e}")
                nc.tensor.matmul(Dzpp2[:], smt["eq"], O2m[:], start=True, stop=True)

                t2e = scr.tile([128, FD], f16, tag="s1_1", name=f"t2e_{e}")
                nc.scalar.activation(t2e[:], z2e[:], AF.Tanh, bias=smt["be2b2"])
                s2e = scr.tile([128, FD], f16, tag="tp1f0", name=f"s2e_{e}")
                TT(nc.vector, s2e[:], t2e[:], t2e[:])
                w_ = scr.tile([128, FD], f16, tag="tp1f1", name=f"w_{e}")
                nc.vector.tensor_scalar(w_[:], s2e[:], -1.0, 1.0, ALU.mult, ALU.add)
                z2ep16 = scr.tile([128, FD], f16, tag="z2ep16", name=f"z2ep16_{e}")
                nc.scalar.copy(z2ep16[:], z2ep[:])
                z2ep2 = scr.tile([128, FD], f16, tag="z2ep2", name=f"z2ep2_{e}")
                TT(nc.gpsimd, z2ep2[:], z2ep16[:], z2ep16[:])
                Q1 = scr.tile([128, FD], f16, tag="Q1", name=f"Q1_{e}")
                TT(nc.gpsimd, Q1[:], t2e[:], w_[:])
                wpc = scr.tile([128, FD], f16, tag="wpc", name=f"wpc_{e}")
                TT(nc.vector, wpc[:], Q1[:], z2ep16[:])
                R1 = scr.tile([128, FD], f16, tag="R1", name=f"R1_{e}")
                nc.vector.scalar_tensor_tensor(
                    R1[:], s2e[:], 1.0 / 3.0, w_[:], ALU.subtract, ALU.mult)
                R2f = scr.tile([128, FD], f16, tag="R2f", name=f"R2f_{e}")
                TT(nc.gpsimd, R2f[:], R1[:], z2ep2[:])
                R3f = scr.tile([128, FD], f16, tag="R3f", name=f"R3f_{e}")
                TT(nc.vector, R3f[:], Q1[:], z2epp[:])
                t1m = scr.tile([128, FD], f16, tag="t1m", name=f"t1m_{e}")
                nc.vector.scalar_tensor_tensor(
                    t1m[:], R2f[:], 3.0, R3f[:], ALU.mult, ALU.subtract)
                F1 = scr.tile([128, FD], f16, tag="F1", name=f"F1_{e}")
                TT(nc.vector, F1[:], t1m[:], Dz[:])
                DyNs = scr.tile([128, FD], f16, tag="DyNs", name=f"DyNs_{e}")
                nc.scalar.copy(DyNs[:], DyN[:])
                t2m = scr.tile([128, FD], f16, tag="t2m", name=f"t2m_{e}")
                nc.vector.scalar_tensor_tensor(
                    t2m[:], DzpN[:], 4.0, DyNs[:], ALU.mult, ALU.add)
                F2 = scr.tile([128, FD], f16, tag="F2", name=f"F2_{e}")
                TT(nc.gpsimd, F2[:], wpc[:], t2m[:])
                DypNs = scr.tile([128, FD], f16, tag="DypNs", name=f"DypNs_{e}")
                nc.scalar.copy(DypNs[:], DypN[:])
                t3m = scr.tile([128, FD], f16, tag="t3m", name=f"t3m_{e}")
                TT(nc.vector, t3m[:], Dzpp2[:], DypNs[:], ALU.add)
                F3 = scr.tile([128, FD], f16, tag="F3", name=f"F3_{e}")
                TT(nc.vector, F3[:], w_[:], t3m[:])

                vps = ps.tile([2, FD], f32, tag=trio[1], name=f"vps_{e}")
                nc.tensor.matmul(vps[:], smt["v6"][:, 0:2], F1[:], start=True, stop=False)
                nc.tensor.matmul(vps[:], smt["v6"][:, 2:4], F2[:], start=False, stop=False)
                nc.tensor.matmul(vps[:], smt["v6"][:, 4:6], F3[:], start=False, stop=True)
                ot = scr.tile([2, FD], f32, tag="ot", name=f"ot_{e}")
                nc.scalar.copy(ot[:], vps[:])
                nc.sync.dma_start(out_d.ap()[:, e * FD:(e + 1) * FD], ot[:])

    nc.compile()
    return nc


def _get_nc():
    if "nc" not in _CACHE:
        _CACHE["nc"] = _build()
    return _CACHE["nc"]


def kernel(**inputs):
    import concourse.bass_utils as bass_utils

    f = lambda k: np.asarray(inputs[k], np.float32)
    a, x, t = f("a"), f("x"), np.float32(inputs["t"])
    Wb, Wt1, bt1, Wt2, bt2 = f("Wb"), f("Wt1"), f("bt1"), f("Wt2"), f("bt2")
    Wt3, We1, be1, We2, be2, We3 = (
        f("Wt3"), f("We1"), f("be1"), f("We2"), f("be2"), f("We3"))

    h16 = lambda v: np.asarray(v, np.float32).astype(np.float16)
    def pair16(v):
        h = h16(v)
        return h, h16(np.asarray(v, np.float32) - h.astype(np.float32))

    w1 = Wt1[:, 0]
    c1b = (Wt1[:, 1] * t + bt1)[:, None]
    w1h, w1l = pair16(w1)
    w11 = np.stack([w1h, w1h, w1l, w1l])                       # [4,128]
    wt2t = np.ascontiguousarray(Wt2.T)
    wt2h, wt2l = pair16(wt2t)
    w2ah, w2al = pair16(wt2t * w1[:, None])
    w2b = h16(wt2t * (-2.0 * w1 ** 2)[:, None])
    w2c = h16(wt2t * (6.0 * w1 ** 3)[:, None])
    wt3h, wt3l = pair16(Wt3)

    p, q, v = We1[:, 0], We1[:, 1], We3[0]
    ph = h16(p)
    qh = h16(q)
    # mov12 rows: 0 uhA, 1 ulA, 2 uhB, 3 ulB, 4 uxhA, 5 uxlA, 6 uxhB, 7 uxlB,
    #             8 uxxA, 9 uxxB, 10 uxxxA, 11 uxxxB
    A_, B_ = slice(0, 64), slice(64, 128)
    def stat12(rows):
        S = np.zeros((12, 128), np.float16)
        for r, vec, cs in rows:
            S[r, cs] = vec
        return S
    SEH = stat12([(0, ph, A_), (1, ph, A_), (2, ph, B_), (3, ph, B_),
                  (4, qh, A_), (5, qh, A_), (6, qh, B_), (7, qh, B_)])
    SPH = stat12([(4, ph, A_), (5, ph, A_), (6, ph, B_), (7, ph, B_),
                  (8, qh, A_), (9, qh, B_)])
    SPPH = stat12([(8, ph, A_), (9, ph, B_), (10, qh, A_), (11, qh, B_)])

    blk = lambda M: np.block([[M, np.zeros_like(M)], [np.zeros_like(M), M]])
    We2T = We2.T
    e0 = h16(blk(We2T))
    eq = h16(blk(We2T * q[:, None]))
    ep = h16(blk(We2T * p[:, None]))
    v6 = np.zeros((128, 6), np.float16)
    for i in range(3):
        v6[0:64, 2 * i] = h16(2.0 * v)
        v6[64:128, 2 * i + 1] = h16(2.0 * v)
    sel4m = np.zeros((8, 4), np.float32)
    for j in range(4):
        sel4m[2 * j, j] = 1.0
        sel4m[2 * j + 1, j] = 1.0

    pkb = np.zeros((128, PKB_COLS), np.float16)
    for n_, arr in [("wt2h", wt2h), ("wt2l", wt2l), ("w2ah", w2ah),
                    ("w2al", w2al), ("w2b", w2b), ("w2c", w2c)]:
        pkb[:, _PKB[n_]:_PKB[n_] + 128] = arr
    pkc = np.zeros((128, PKC_COLS), np.float16)
    for n_, arr in [("wt3h", wt3h), ("wt3l", wt3l), ("e0", e0), ("eq", eq),
                    ("ep", ep)]:
        pkc[:, _PKC[n_]:_PKC[n_] + 128] = arr
    for n_, arr in [("SEH", SEH), ("SPH", SPH), ("SPPH", SPPH)]:
        pkc[0:12, _PKC[n_]:_PKC[n_] + 128] = arr
    pkc[:, _PKC["v6"]:_PKC["v6"] + 6] = v6
    pk32 = np.zeros((128, PK32_COLS), np.float32)
    pk32[:, 0] = c1b[:, 0]
    pk32[:, 1] = bt2
    pk32[:, 2] = np.concatenate([be1, be1])
    pk32[:, 3] = np.concatenate([be2, be2])
    pk32[0:8, 4:8] = sel4m

    smalls = {
        "w11": np.ascontiguousarray(w11),
        "pkb": np.ascontiguousarray(pkb),
        "pkc": np.ascontiguousarray(pkc),
        "pk32": np.ascontiguousarray(pk32),
    }

    in_maps = []
    for c in range(NCORES):
        blk_w = Wb[:, c * KSH:(c + 1) * KSH]                   # [128, 65536]
        tr = blk_w.T.reshape(NKT, 128, 128).transpose(1, 0, 2)  # [k1, kt, p]
        tr = tr.reshape(128, NCHUNK, KTC * 128).transpose(1, 0, 2)
        wsh = np.ascontiguousarray(h16(1024.0 * tr))           # [16,128,4096]
        ash = (a[c * KSH:(c + 1) * KSH] / 1024.0).reshape(NKT, 128).T  # [k1, kt]
        ah, al = pair16(ash)
        a2 = np.ascontiguousarray(np.stack([ah, al], axis=2))  # [128,512,2]
        xs = x[c * NPTS:(c + 1) * NPTS]
        xh, xl = pair16(xs)
        x4 = np.ascontiguousarray(np.stack([xh, xl, xh, xl]))  # [4,4096]
        im = {"w": wsh, "a2": a2, "x4": x4}
        im.update(smalls)
        in_maps.append(im)

    global _last_in_maps
    _last_in_maps = in_maps
    nc = _get_nc()
    res = bass_utils.run_bass_kernel_spmd(nc, in_maps, core_ids=list(range(NCORES)))
    outs = []
    for c in range(NCORES):
        o = res.results[c]["out"]          # [2, NPTS//2]
        outs.append(np.asarray(o).reshape(-1))
    return np.concatenate(outs).astype(np.float32)


# revision 5
# speedup vs baseline: 1.2135x; 1.0239x over previous
"""Bass/Trainium2 kernel for nn_HNO_37065567764989 (self-contained).

Strategy (8 NeuronCores, SPMD):
- Branch matvec b = Wb@a column-sharded 8 ways. Each core streams its 16MB
  shard as fp16 (W scaled by 2^10; a as an fp16 hi/lo stationary pair), two
  512KB DMAs per 1MB chunk across queues. 512B AllReduce combines partials.
- Nx=32768 points sharded 8 ways (4096/core). Trunk runs as 4 wide pairs
  (tiles f and f+4 share [128,1024] elementwise ops that write the energy
  movings directly). GpSimd carries only early-pair products plus the
  collective, so the mesh wait never blocks the trunk tail.
- EnergyNet first layer uses runtime outer-product stationaries S=c(x)p,
  c(x)q built on-device after the AllReduce -- no per-row extraction.
- Precision: t2/P1 flow as fp16 hi/lo pairs; t1/tp1, B/C stationaries and
  all product chains are single fp16 (mirror-validated 1.24e-2).
"""
import sys

for _p in ("/opt/trn_rl_repo",):
    if _p not in sys.path:
        sys.path.insert(0, _p)

import numpy as np

MP1, NX, P, HT, HE = 524288, 32768, 128, 128, 64
NCORES = 8
KSH = MP1 // NCORES        # 65536 contraction elems per core
NKT = KSH // 128           # 512 k-tiles
NCHUNK = 16
KTC = NKT // NCHUNK        # 32 k-tiles per chunk
NPTS = NX // NCORES        # 4096 points per core
FD = 512
WFD = 2 * FD               # wide pair width
NTRUNK = NPTS // FD        # 8 trunk tiles
NEN = NTRUNK // 2          # 4 energy tiles / trunk pairs

_PKB = {"wt2h": 0, "wt2l": 128, "w2ah": 256, "w2al": 384, "w2b": 512,
        "w2c": 640}
PKB_COLS = 768
_PKC = {"wt3h": 0, "wt3l": 128, "e0": 256, "eq": 384, "ep": 512,
        "pq2": 640, "v6": 768}
PKC_COLS = 774
_PK32 = {"c1b": 0, "bt2b": 1, "be1b2": 2, "be2b2": 3, "sel4m": 4}
PK32_COLS = 8

_CACHE = {}


def _build():
    import concourse.bacc as bacc
    import concourse.mybir as mybir
    from concourse import tile

    f32 = mybir.dt.float32
    f16 = mybir.dt.float16
    AF = mybir.ActivationFunctionType
    ALU = mybir.AluOpType

    nc = bacc.Bacc("TRN2", target_bir_lowering=False, debug=False,
                   num_devices=NCORES)

    w_d = nc.dram_tensor("w", [NCHUNK, 128, KTC * 128], f16, kind="ExternalInput")
    a_d = nc.dram_tensor("a2", [128, NKT, 2], f16, kind="ExternalInput")
    x_d = nc.dram_tensor("x4", [4, NPTS], f16, kind="ExternalInput")
    w11_d = nc.dram_tensor("w11", [4, 128], f16, kind="ExternalInput")
    pkb_d = nc.dram_tensor("pkb", [128, PKB_COLS], f16, kind="ExternalInput")
    pkc_d = nc.dram_tensor("pkc", [128, PKC_COLS], f16, kind="ExternalInput")
    pk32_d = nc.dram_tensor("pk32", [128, PK32_COLS], f32, kind="ExternalInput")
    out_d = nc.dram_tensor("out", [2, NPTS // 2], f32, kind="ExternalOutput")
    cc_in = nc.dram_tensor("cc_in", [128, 1], f32)
    cc_out = nc.dram_tensor("cc_out", [128, 1], f32, addr_space="Shared")

    def TT(eng, out, i0, i1, op=ALU.mult):
        eng.tensor_tensor(out, i0, i1, op)

    with tile.TileContext(nc) as tc:
        with (
            tc.tile_pool(name="smp", bufs=1) as smp,
            tc.tile_pool(name="persist", bufs=1) as persist,
            tc.tile_pool(name="wpool", bufs=4) as wpool,
            tc.tile_pool(name="scr", bufs=1) as scr,
            tc.tile_pool(name="ps8", bufs=1, space="PSUM") as ps,
        ):
            # ---- packed constant loads (6 DMA issues) ----
            x4 = smp.tile([4, NPTS], f16, name="x4t")
            nc.sync.dma_start(x4[:], x_d.ap())
            w11 = smp.tile([4, 128], f16, name="w11t")
            nc.sync.dma_start(w11[:], w11_d.ap())
            pk32 = smp.tile([128, PK32_COLS], f32, name="pk32t")
            nc.sync.dma_start(pk32[:], pk32_d.ap())
            a2 = smp.tile([128, NKT, 2], f16, name="a2t")
            nc.sync.dma_start(a2[:], a_d.ap())
            pkb = smp.tile([128, PKB_COLS], f16, name="pkbt")
            nc.sync.dma_start(pkb[:], pkb_d.ap())
            pkc = smp.tile([128, PKC_COLS], f16, name="pkct")
            nc.sync.dma_start(pkc[:], pkc_d.ap())

            smt = {"w11": w11[:]}
            for n_, c0 in _PKB.items():
                smt[n_] = pkb[:, c0:c0 + 128]
            for n_, c0 in _PKC.items():
                if n_ == "v6":
                    smt[n_] = pkc[:, c0:c0 + 6]
                elif n_ == "pq2":
                    smt[n_] = pkc[0:1, c0:c0 + 128]
                else:
                    smt[n_] = pkc[:, c0:c0 + 128]
            for n_, c0 in _PK32.items():
                if n_ == "sel4m":
                    smt[n_] = pk32[0:8, c0:c0 + 4]
                else:
                    smt[n_] = pk32[:, c0:c0 + 1]

            # ---- trunk layer-1 z1 matmuls (pairs f, f+4) ----
            zpair = [("zA", "zB"), ("aA", "aB")]
            z1ps = {}
            for j in range(NEN):
                tA, tB = zpair[j % 2]
                for hx, f in enumerate((j, j + 4)):
                    cs = slice(f * FD, (f + 1) * FD)
                    z1 = ps.tile([128, FD], f32, tag=(tA, tB)[hx], name=f"z1_{f}")
                    nc.tensor.matmul(z1[:], smt["w11"], x4[:, cs], start=True,
                                     stop=True)
                    z1ps[f] = z1

            # ---- trunk layer-1 elementwise (wide pairs, single-fp16 t1/tp1) --
            l1 = {}
            for j in range(NEN):
                t1f = scr.tile([128, WFD], f32, tag="t1f", name=f"t1f_{j}")
                for hx, f in enumerate((j, j + 4)):
                    hs = slice(hx * FD, (hx + 1) * FD)
                    nc.scalar.activation(t1f[:, hs], z1ps[f][:], AF.Tanh,
                                         bias=smt["c1b"])
                t1h = persist.tile([128, WFD], f16, tag=f"t1h_{j % 2}", name=f"t1h_{j}")
                nc.scalar.copy(t1h[:], t1f[:])
                s1 = scr.tile([128, WFD], f32, tag="s1", name=f"s1_{j}")
                nc.scalar.square(s1[:], t1f[:])
                tp1f = scr.tile([128, WFD], f32, tag="tp1f", name=f"tp1f_{j}")
                nc.scalar.activation(tp1f[:], s1[:], AF.Copy, bias=1.0, scale=-1.0)
                tp1h = persist.tile([128, WFD], f16, tag=f"tp1h_{j % 2}", name=f"tp1h_{j}")
                nc.scalar.copy(tp1h[:], tp1f[:])
                g2m = persist.tile([128, WFD], f16, tag=f"g2m_{j % 2}", name=f"g2m_{j}")
                TT(nc.vector, g2m[:], t1f[:], tp1f[:])
                g3m = persist.tile([128, WFD], f16, tag=f"g3m_{j % 2}", name=f"g3m_{j}")
                nc.vector.scalar_tensor_tensor(
                    g3m[:], s1[:], 1.0 / 3.0, tp1f[:], ALU.subtract, ALU.mult)
                l1[j] = (t1h, tp1h, g2m, g3m)

            # ---- matvec: stream W shard (2 DMA splits per 1MB chunk) ----
            b8 = ps.tile([8, FD], f32, tag="pMV", name="b8")
            half = KTC * 64
            for i in range(NCHUNK):
                wch = wpool.tile([128, KTC * 128], f16, tag="wch", name="wch")
                nc.sync.dma_start(wch[:, 0:half], w_d.ap()[i][:, 0:half])
                nc.sync.dma_start(wch[:, half:], w_d.ap()[i][:, half:])
                for g in range(KTC // 4):
                    nc.tensor.matmul(
                        b8[:], a2[:, i * KTC + 4 * g:i * KTC + 4 * (g + 1), :],
                        wch[:, g * 512:(g + 1) * 512],
                        start=(i == 0 and g == 0),
                        stop=(i == NCHUNK - 1 and g == KTC // 4 - 1),
                    )

            # ---- local reduce + AllReduce ----
            b8sb = smp.tile([8, FD], f32, name="b8sb")
            nc.scalar.copy(b8sb[:], b8[:])
            bcol = ps.tile([128, 1], f32, tag="pBC", name="bcol")
            for j in range(4):
                nc.tensor.matmul(bcol[:], b8sb[:, j * 128:(j + 1) * 128],
                                 smt["sel4m"][:, j:j + 1],
                                 start=(j == 0), stop=(j == 3))
            b_loc = smp.tile([128, 1], f32, name="bloc")
            nc.scalar.copy(b_loc[:], bcol[:])
            nc.sync.dma_start(cc_in.ap(), b_loc[:])
            nc.gpsimd.collective_compute(
                "AllReduce", ALU.add,
                replica_groups=[list(range(NCORES))],
                ins=[cc_in.ap()], outs=[cc_out.ap()],
            )
            b_ar = smp.tile([128, 1], f32, name="bar")
            nc.sync.dma_start(b_ar[:], cc_out.ap())

            # ---- trunk layer-2 wave (wide pairs) ----
            sh = {}
            for j in range(NEN):
                t1h, tp1h, g2m, g3m = l1[j]
                sh[j] = tuple(
                    persist.tile([128, WFD], f16, tag=f"sh{nm}_{j}",
                                 name=f"sh_{nm}_{j}")
                    for nm in ("t2h", "t2l", "P1h", "P1l", "ux2", "ux3"))
                t2h_s, t2l_s, P1h_s, P1l_s, ux2_s, ux3_s = sh[j]
                ge = nc.gpsimd if j < 2 else nc.vector

                zw, aw = [], []
                for hx in range(2):
                    hs = slice(hx * FD, (hx + 1) * FD)
                    z2 = ps.tile([128, FD], f32, tag=zpair[0][hx], name=f"z2_{j}{hx}")
                    nc.tensor.matmul(z2[:], smt["wt2h"], t1h[:, hs], start=True,
                                     stop=False)
                    nc.tensor.matmul(z2[:], smt["wt2l"], t1h[:, hs], start=False,
                                     stop=True)
                    zw.append(z2)
                    A = ps.tile([128, FD], f32, tag=zpair[1][hx], name=f"A_{j}{hx}")
                    nc.tensor.matmul(A[:], smt["w2ah"], tp1h[:, hs], start=True,
                                     stop=False)
                    nc.tensor.matmul(A[:], smt["w2al"], tp1h[:, hs], start=False,
                                     stop=True)
                    aw.append(A)

                t2f = scr.tile([128, WFD], f32, tag="t2f", name=f"t2f_{j}")
                Bc = scr.tile([128, WFD], f16, tag="Bc", name=f"Bc_{j}")
                Cc = scr.tile([128, WFD], f16, tag="Cc", name=f"Cc_{j}")
                A2c = scr.tile([128, WFD], f16, tag="A2c", name=f"A2c_{j}")
                Acp = scr.tile([128, WFD], f16, tag="Acp", name=f"Acp_{j}")
                for hx in range(2):
                    hs = slice(hx * FD, (hx + 1) * FD)
                    nc.scalar.activation(t2f[:, hs], zw[hx][:], AF.Tanh,
                                         bias=smt["bt2b"])
                    nc.scalar.square(A2c[:, hs], aw[hx][:])
                    nc.scalar.copy(Acp[:, hs], aw[hx][:])
                    B = ps.tile([128, FD], f32, tag="pB", name=f"B_{j}{hx}")
                    nc.tensor.matmul(B[:], smt["w2b"], g2m[:, hs], start=True,
                                     stop=True)
                    nc.scalar.copy(Bc[:, hs], B[:])
                    C = ps.tile([128, FD], f32, tag="pC", name=f"C_{j}{hx}")
                    nc.tensor.matmul(C[:], smt["w2c"], g3m[:, hs], start=True,
                                     stop=True)
                    nc.scalar.copy(Cc[:, hs], C[:])

                nc.scalar.copy(t2h_s[:], t2f[:])
                TT(nc.vector, t2l_s[:], t2f[:], t2h_s[:], ALU.subtract)
                s2 = scr.tile([128, WFD], f32, tag="s2", name=f"s2_{j}")
                nc.scalar.square(s2[:], t2f[:])
                tp2 = scr.tile([128, WFD], f32, tag="tp2", name=f"tp2_{j}")
                nc.vector.tensor_scalar(tp2[:], s2[:], -1.0, 1.0, ALU.mult, ALU.add)
                tp2c = scr.tile([128, WFD], f16, tag="tp2c", name=f"tp2c_{j}")
                nc.scalar.activation(tp2c[:], s2[:], AF.Copy, bias=1.0, scale=-1.0)
                P1f = scr.tile([128, WFD], f32, tag="P1f", name=f"P1f_{j}")
                for hx in range(2):
                    hs = slice(hx * FD, (hx + 1) * FD)
                    TT(nc.vector, P1f[:, hs], tp2[:, hs], aw[hx][:])
                nc.scalar.copy(P1h_s[:], P1f[:])
                TT(nc.vector, P1l_s[:], P1f[:], P1h_s[:], ALU.subtract)

                T1 = scr.tile([128, WFD], f16, tag="T1", name=f"T1_{j}")
                TT(ge, T1[:], t2h_s[:], A2c[:])
                E = scr.tile([128, WFD], f16, tag="E", name=f"E_{j}")
                nc.vector.scalar_tensor_tensor(
                    E[:], T1[:], -2.0, Bc[:], ALU.mult, ALU.add)
                TT(nc.vector, ux2_s[:], tp2c[:], E[:])
                A3 = scr.tile([128, WFD], f16, tag="A3", name=f"A3_{j}")
                TT(ge, A3[:], A2c[:], Acp[:])
                G1 = scr.tile([128, WFD], f16, tag="G1", name=f"G1_{j}")
                nc.vector.scalar_tensor_tensor(
                    G1[:], tp2c[:], 2.0 / 3.0, A3[:], ALU.subtract, ALU.mult)
                G2 = scr.tile([128, WFD], f16, tag="G2", name=f"G2_{j}")
                TT(ge, G2[:], t2h_s[:], Acp[:])
                G3 = scr.tile([128, WFD], f16, tag="G3", name=f"G3_{j}")
                TT(ge, G3[:], G2[:], Bc[:])
                D = scr.tile([128, WFD], f16, tag="Dd", name=f"D_{j}")
                TT(nc.vector, D[:], G1[:], G3[:], ALU.add)
                H = scr.tile([128, WFD], f16, tag="Hh", name=f"H_{j}")
                nc.vector.scalar_tensor_tensor(
                    H[:], D[:], -6.0, Cc[:], ALU.mult, ALU.add)
                TT(nc.vector, ux3_s[:], tp2c[:], H[:])

            # ---- b -> c -> outer-product stationaries S = c(x)p, c(x)q ----
            b16 = smp.tile([128, 1], f16, name="b16")
            nc.scalar.copy(b16[:], b_ar[:])
            c0p = ps.tile([1, 128], f32, tag="pBC", name="c0p")
            nc.tensor.matmul(c0p[:], b16[:], smt["wt3h"], start=True, stop=False)
            nc.tensor.matmul(c0p[:], b16[:], smt["wt3l"], start=False, stop=True)
            c0sb = smp.tile([1, 128], f16, name="c0sb")
            nc.scalar.copy(c0sb[:], c0p[:])
            scpq_p = ps.tile([128, 128], f32, tag="pBC", name="scpq_p")
            nc.tensor.matmul(scpq_p[:], c0sb[:], smt["pq2"], start=True, stop=True)
            Scpq = smp.tile([128, 128], f16, name="Scpq")
            nc.scalar.copy(Scpq[:], scpq_p[:])
            Sp, Sq = Scpq[:, 0:64], Scpq[:, 64:128]

            # ---- energy phase ----
            for e in range(NEN):
                t2h_s, t2l_s, P1h_s, P1l_s, ux2_s, ux3_s = sh[e]
                trio = [["zA", "zB", "aA"], ["aB", "pB", "pC"]][e % 2]
                dzt, dyt = ("pBC", "pMV") if e % 2 == 0 else ("pMV", "pBC")

                z1e = ps.tile([128, FD], f32, tag=trio[0], name=f"z1e_{e}")
                z1p = ps.tile([128, FD], f32, tag=trio[1], name=f"z1p_{e}")
                z1pp = ps.tile([128, FD], f32, tag=trio[2], name=f"z1pp_{e}")
                for hx in range(2):
                    hs = slice(hx * FD, (hx + 1) * FD)
                    rs = slice(hx * 64, (hx + 1) * 64)
                    nc.tensor.matmul(z1e[rs, :], Sp, t2h_s[:, hs], start=True,
                                     stop=False)
                    nc.tensor.matmul(z1e[rs, :], Sp, t2l_s[:, hs], start=False,
                                     stop=False)
                    nc.tensor.matmul(z1e[rs, :], Sq, P1h_s[:, hs], start=False,
                                     stop=False)
                    nc.tensor.matmul(z1e[rs, :], Sq, P1l_s[:, hs], start=False,
                                     stop=True)
                    nc.tensor.matmul(z1p[rs, :], Sp, P1h_s[:, hs], start=True,
                                     stop=False)
                    nc.tensor.matmul(z1p[rs, :], Sp, P1l_s[:, hs], start=False,
                                     stop=False)
                    nc.tensor.matmul(z1p[rs, :], Sq, ux2_s[:, hs], start=False,
                                     stop=True)
                    nc.tensor.matmul(z1pp[rs, :], Sp, ux2_s[:, hs], start=True,
                                     stop=False)
                    nc.tensor.matmul(z1pp[rs, :], Sq, ux3_s[:, hs], start=False,
                                     stop=True)

                t1ef = scr.tile([128, FD], f32, tag="t1ef", name=f"t1ef_{e}")
                nc.scalar.activation(t1ef[:], z1e[:], AF.Tanh, bias=smt["be1b2"])
                t1eh = scr.tile([128, FD], f16, tag="t1eh", name=f"t1eh_{e}")
                nc.scalar.copy(t1eh[:], t1ef[:])
                z1psb = scr.tile([128, FD], f16, tag="z1psb", name=f"z1psb_{e}")
                nc.scalar.copy(z1psb[:], z1p[:])
                z1ppsb = scr.tile([128, FD], f16, tag="z1ppsb", name=f"z1ppsb_{e}")
                nc.scalar.copy(z1ppsb[:], z1pp[:])
                s1e = scr.tile([128, FD], f16, tag="s1e", name=f"s1e_{e}")
                nc.scalar.square(s1e[:], t1ef[:])
                m_ = scr.tile([128, FD], f16, tag="m_", name=f"m_{e}")
                nc.scalar.activation(m_[:], s1e[:], AF.Copy, bias=1.0, scale=-1.0)
                z1p2 = scr.tile([128, FD], f16, tag="z1p2", name=f"z1p2_{e}")
                TT(nc.gpsimd, z1p2[:], z1psb[:], z1psb[:])
                N1 = scr.tile([128, FD], f16, tag="N1", name=f"N1_{e}")
                TT(nc.vector, N1[:], t1eh[:], m_[:])
                a1p = scr.tile([128, FD], f16, tag="a1p", name=f"a1p_{e}")
                TT(nc.vector, a1p[:], m_[:], z1psb[:])
                N2 = scr.tile([128, FD], f16, tag="N2", name=f"N2_{e}")
                TT(nc.gpsimd, N2[:], N1[:], z1p2[:])
                N3 = scr.tile([128, FD], f16, tag="N3", name=f"N3_{e}")
                TT(nc.vector, N3[:], m_[:], z1ppsb[:])
                zin = scr.tile([128, FD], f16, tag="zin", name=f"zin_{e}")
                nc.vector.scalar_tensor_tensor(
                    zin[:], N2[:], -2.0, N3[:], ALU.mult, ALU.add)
                mpc = scr.tile([128, FD], f16, tag="mpc", name=f"mpc_{e}")
                TT(nc.vector, mpc[:], N1[:], z1psb[:])
                O1 = scr.tile([128, FD], f16, tag="O1", name=f"O1_{e}")
                nc.vector.scalar_tensor_tensor(
                    O1[:], s1e[:], 1.0 / 3.0, m_[:], ALU.subtract, ALU.mult)
                O2f = scr.tile([128, FD], f16, tag="O2f", name=f"O2f_{e}")
                TT(nc.gpsimd, O2f[:], O1[:], z1p2[:])
                O3f = scr.tile([128, FD], f16, tag="O3f", name=f"O3f_{e}")
                TT(nc.vector, O3f[:], N1[:], z1ppsb[:])
                O2m = scr.tile([128, FD], f16, tag="O2m", name=f"O2m_{e}")
                nc.vector.scalar_tensor_tensor(
                    O2m[:], O2f[:], 3.0, O3f[:], ALU.mult, ALU.subtract)

                z2e = ps.tile([128, FD], f32, tag=trio[0], name=f"z2e_{e}")
                nc.tensor.matmul(z2e[:], smt["e0"], t1eh[:], start=True, stop=True)
                z2ep = ps.tile([128, FD], f32, tag=trio[1], name=f"z2ep_{e}")
                nc.tensor.matmul(z2ep[:], smt["e0"], a1p[:], start=True, stop=True)
                z2epp = ps.tile([128, FD], f32, tag=trio[2], name=f"z2epp_{e}")
                nc.tensor.matmul(z2epp[:], smt["e0"], zin[:], start=True, stop=True)
                Dz = ps.tile([128, FD], f32, tag=dzt, name=f"Dz_{e}")
                nc.tensor.matmul(Dz[:], smt["eq"], m_[:], start=True, stop=True)
                DyN = ps.tile([128, FD], f32, tag=dyt, name=f"DyN_{e}")
                nc.tensor.matmul(DyN[:], smt["ep"], m_[:], start=True, stop=True)
                DzpN = ps.tile([128, FD], f32, tag=trio[0], name=f"DzpN_{e}")
                nc.tensor.matmul(DzpN[:], smt["eq"], mpc[:], start=True, stop=True)
                DypN = ps.tile([128, FD], f32, tag=trio[1], name=f"DypN_{e}")
                nc.tensor.matmul(DypN[:], smt["ep"], mpc[:], start=True, stop=True)
                Dzpp2 = ps.tile([128, FD], f32, tag=trio[2], name=f"Dzpp2_{e}")
                nc.tensor.matmul(Dzpp2[:], smt["eq"], O2m[:], start=True, stop=True)

                t2e = scr.tile([128, FD], f16, tag="t2e", name=f"t2e_{e}")
                nc.scalar.activation(t2e[:], z2e[:], AF.Tanh, bias=smt["be2b2"])
                s2e = scr.tile([128, FD], f16, tag="s2e", name=f"s2e_{e}")
                nc.scalar.square(s2e[:], t2e[:])
                w_ = scr.tile([128, FD], f16, tag="w_", name=f"w_{e}")
                nc.scalar.activation(w_[:], s2e[:], AF.Copy, bias=1.0, scale=-1.0)
                z2ep16 = scr.tile([128, FD], f16, tag="z2ep16", name=f"z2ep16_{e}")
                nc.scalar.copy(z2ep16[:], z2ep[:])
                z2ep2 = scr.tile([128, FD], f16, tag="z2ep2", name=f"z2ep2_{e}")
                TT(nc.gpsimd, z2ep2[:], z2ep16[:], z2ep16[:])
                Q1 = scr.tile([128, FD], f16, tag="Q1", name=f"Q1_{e}")
                TT(nc.vector, Q1[:], t2e[:], w_[:])
                wpc = scr.tile([128, FD], f16, tag="wpc", name=f"wpc_{e}")
                TT(nc.vector, wpc[:], Q1[:], z2ep16[:])
                R1 = scr.tile([128, FD], f16, tag="R1", name=f"R1_{e}")
                nc.vector.scalar_tensor_tensor(
                    R1[:], s2e[:], 1.0 / 3.0, w_[:], ALU.subtract, ALU.mult)
                R2f = scr.tile([128, FD], f16, tag="R2f", name=f"R2f_{e}")
                TT(nc.gpsimd, R2f[:], R1[:], z2ep2[:])
                R3f = scr.tile([128, FD], f16, tag="R3f", name=f"R3f_{e}")
                TT(nc.vector, R3f[:], Q1[:], z2epp[:])
                t1m = scr.tile([128, FD], f16, tag="t1m", name=f"t1m_{e}")
                nc.vector.scalar_tensor_tensor(
                    t1m[:], R2f[:], 3.0, R3f[:], ALU.mult, ALU.subtract)
                F1 = scr.tile([128, FD], f16, tag="F1", name=f"F1_{e}")
                TT(nc.vector, F1[:], t1m[:], Dz[:])
                DyNs = scr.tile([128, FD], f16, tag="DyNs", name=f"DyNs_{e}")
                nc.scalar.copy(DyNs[:], DyN[:])
                t2m = scr.tile([128, FD], f16, tag="t2m", name=f"t2m_{e}")
                nc.vector.scalar_tensor_tensor(
                    t2m[:], DzpN[:], 4.0, DyNs[:], ALU.mult, ALU.add)
                F2 = scr.tile([128, FD], f16, tag="F2", name=f"F2_{e}")
                TT(nc.gpsimd, F2[:], wpc[:], t2m[:])
                DypNs = scr.tile([128, FD], f16, tag="DypNs", name=f"DypNs_{e}")
                nc.scalar.copy(DypNs[:], DypN[:])
                t3m = scr.tile([128, FD], f16, tag="t3m", name=f"t3m_{e}")
                TT(nc.vector, t3m[:], Dzpp2[:], DypNs[:], ALU.add)
                F3 = scr.tile([128, FD], f16, tag="F3", name=f"F3_{e}")
                TT(nc.vector, F3[:], w_[:], t3m[:])

                vps = ps.tile([2, FD], f32, tag=trio[1], name=f"vps_{e}")
                nc.tensor.matmul(vps[:], smt["v6"][:, 0:2], F1[:], start=True,
                                 stop=False)
                nc.tensor.matmul(vps[:], smt["v6"][:, 2:4], F2[:], start=False,
                                 stop=False)
                nc.tensor.matmul(vps[:], smt["v6"][:, 4:6], F3[:], start=False,
                                 stop=True)
                ot = scr.tile([2, FD], f32, tag="ot", name=f"ot_{e}")
                nc.scalar.copy(ot[:], vps[:])
                nc.sync.dma_start(out_d.ap()[:, e * FD:(e + 1) * FD], ot[:])

    nc.compile()
    return nc


def _get_nc():
    if "nc" not in _CACHE:
        _CACHE["nc"] = _build()
    return _CACHE["nc"]


def kernel(**inputs):
    import concourse.bass_utils as bass_utils

    f = lambda k: np.asarray(inputs[k], np.float32)
    a, x, t = f("a"), f("x"), np.float32(inputs["t"])
    Wb, Wt1, bt1, Wt2, bt2 = f("Wb"), f("Wt1"), f("bt1"), f("Wt2"), f("bt2")
    Wt3, We1, be1, We2, be2, We3 = (
        f("Wt3"), f("We1"), f("be1"), f("We2"), f("be2"), f("We3"))

    h16 = lambda v: np.asarray(v, np.float32).astype(np.float16)
    def pair16(v):
        h = h16(v)
        return h, h16(np.asarray(v, np.float32) - h.astype(np.float32))

    w1 = Wt1[:, 0]
    c1b = (Wt1[:, 1] * t + bt1)[:, None]
    w1h, w1l = pair16(w1)
    w11 = np.stack([w1h, w1h, w1l, w1l])                       # [4,128]
    wt2t = np.ascontiguousarray(Wt2.T)
    wt2h, wt2l = pair16(wt2t)
    w2ah, w2al = pair16(wt2t * w1[:, None])
    w2b = h16(wt2t * (-2.0 * w1 ** 2)[:, None])
    w2c = h16(wt2t * (6.0 * w1 ** 3)[:, None])
    wt3h, wt3l = pair16(Wt3)

    p, q, v = We1[:, 0], We1[:, 1], We3[0]
    pq2 = np.zeros((1, 128), np.float16)
    pq2[0, 0:64] = h16(p)
    pq2[0, 64:128] = h16(q)

    blk = lambda M: np.block([[M, np.zeros_like(M)], [np.zeros_like(M), M]])
    We2T = We2.T
    e0 = h16(blk(We2T))
    eq = h16(blk(We2T * q[:, None]))
    ep = h16(blk(We2T * p[:, None]))
    v6 = np.zeros((128, 6), np.float16)
    for i in range(3):
        v6[0:64, 2 * i] = h16(2.0 * v)
        v6[64:128, 2 * i + 1] = h16(2.0 * v)
    sel4m = np.zeros((8, 4), np.float32)
    for j in range(4):
        sel4m[2 * j, j] = 1.0
        sel4m[2 * j + 1, j] = 1.0

    pkb = np.zeros((128, PKB_COLS), np.float16)
    for n_, arr in [("wt2h", wt2h), ("wt2l", wt2l), ("w2ah", w2ah),
                    ("w2al", w2al), ("w2b", w2b), ("w2c", w2c)]:
        pkb[:, _PKB[n_]:_PKB[n_] + 128] = arr
    pkc = np.zeros((128, PKC_COLS), np.float16)
    for n_, arr in [("wt3h", wt3h), ("wt3l", wt3l), ("e0", e0), ("eq", eq),
                    ("ep", ep)]:
        pkc[:, _PKC[n_]:_PKC[n_] + 128] = arr
    pkc[0:1, _PKC["pq2"]:_PKC["pq2"] + 128] = pq2
    pkc[:, _PKC["v6"]:_PKC["v6"] + 6] = v6
    pk32 = np.zeros((128, PK32_COLS), np.float32)
    pk32[:, 0] = c1b[:, 0]
    pk32[:, 1] = bt2
    pk32[:, 2] = np.concatenate([be1, be1])
    pk32[:, 3] = np.concatenate([be2, be2])
    pk32[0:8, 4:8] = sel4m

    smalls = {
        "w11": np.ascontiguousarray(w11),
        "pkb": np.ascontiguousarray(pkb),
        "pkc": np.ascontiguousarray(pkc),
        "pk32": np.ascontiguousarray(pk32),
    }

    in_maps = []
    for c in range(NCORES):
        blk_w = Wb[:, c * KSH:(c + 1) * KSH]                   # [128, 65536]
        tr = blk_w.T.reshape(NKT, 128, 128).transpose(1, 0, 2)  # [k1, kt, p]
        tr = tr.reshape(128, NCHUNK, KTC * 128).transpose(1, 0, 2)
        wsh = np.ascontiguousarray(h16(1024.0 * tr))           # [16,128,4096]
        ash = (a[c * KSH:(c + 1) * KSH] / 1024.0).reshape(NKT, 128).T  # [k1, kt]
        ah, al = pair16(ash)
        a2 = np.ascontiguousarray(np.stack([ah, al], axis=2))  # [128,512,2]
        xs = x[c * NPTS:(c + 1) * NPTS]
        xh, xl = pair16(xs)
        x4 = np.ascontiguousarray(np.stack([xh, xl, xh, xl]))  # [4,4096]
        im = {"w": wsh, "a2": a2, "x4": x4}
        im.update(smalls)
        in_maps.append(im)

    global _last_in_maps
    _last_in_maps = in_maps
    nc = _get_nc()
    res = bass_utils.run_bass_kernel_spmd(nc, in_maps, core_ids=list(range(NCORES)))
    outs = []
    for c in range(NCORES):
        o = res.results[c]["out"]          # [2, NPTS//2]
        outs.append(np.asarray(o).reshape(-1))
    return np.concatenate(outs).astype(np.float32)


# revision 10
# speedup vs baseline: 1.3102x; 1.0797x over previous
"""Bass/Trainium2 kernel for nn_HNO_37065567764989 (self-contained).

Strategy (8 NeuronCores, SPMD):
- Branch matvec b = Wb@a column-sharded 8 ways. Each core streams its 16MB
  shard as fp16 (W scaled by 2^10; a as an fp16 hi/lo stationary pair), two
  512KB DMAs per 1MB chunk across queues. 512B AllReduce combines partials.
- Nx=32768 points sharded 8 ways (4096/core). Trunk runs as 4 wide pairs
  (tiles f and f+4 share [128,1024] elementwise ops that write the energy
  movings directly). GpSimd carries only early-pair products plus the
  collective, so the mesh wait never blocks the trunk tail.
- EnergyNet first layer uses runtime outer-product stationaries S=c(x)p,
  c(x)q built on-device after the AllReduce -- no per-row extraction.
- Precision: t2/P1 flow as fp16 hi/lo pairs; t1/tp1, B/C stationaries and
  all product chains are single fp16 (mirror-validated 1.24e-2).
"""
import sys

for _p in ("/opt/trn_rl_repo",):
    if _p not in sys.path:
        sys.path.insert(0, _p)

import numpy as np

MP1, NX, P, HT, HE = 524288, 32768, 128, 128, 64
NCORES = 8
KSH = MP1 // NCORES        # 65536 contraction elems per core
NKT = KSH // 128           # 512 k-tiles
NCHUNK = 16
KTC = NKT // NCHUNK        # 32 k-tiles per chunk
NPTS = NX // NCORES        # 4096 points per core
FD = 512
WFD = 2 * FD               # wide pair width
NTRUNK = NPTS // FD        # 8 trunk tiles
NEN = NTRUNK // 2          # 4 energy tiles / trunk pairs

_PKB = {"wt2h": 0, "wt2l": 128, "w2ah": 256, "w2al": 384, "w2b": 512,
        "w2c": 640}
PKB_COLS = 768
_PKC = {"wt3h": 0, "wt3l": 128, "e0": 256, "eq": 384, "ep": 512,
        "pq2": 640, "v6": 768}
PKC_COLS = 774
_PK32 = {"c1b": 0, "bt2b": 1, "be1b2": 2, "be2b2": 3, "sel4m": 4}
PK32_COLS = 8

_CACHE = {}


def _build():
    import concourse.bacc as bacc
    import concourse.mybir as mybir
    from concourse import tile

    f32 = mybir.dt.float32
    f16 = mybir.dt.float16
    AF = mybir.ActivationFunctionType
    ALU = mybir.AluOpType

    nc = bacc.Bacc("TRN2", target_bir_lowering=False, debug=False,
                   num_devices=NCORES)

    w_d = nc.dram_tensor("w", [NCHUNK, 128, KTC * 128], f16, kind="ExternalInput")
    a_d = nc.dram_tensor("a2", [128, NKT, 2], f16, kind="ExternalInput")
    x_d = nc.dram_tensor("x4", [4, NPTS], f16, kind="ExternalInput")
    w11_d = nc.dram_tensor("w11", [4, 128], f16, kind="ExternalInput")
    pkb_d = nc.dram_tensor("pkb", [128, PKB_COLS], f16, kind="ExternalInput")
    pkc_d = nc.dram_tensor("pkc", [128, PKC_COLS], f16, kind="ExternalInput")
    pk32_d = nc.dram_tensor("pk32", [128, PK32_COLS], f32, kind="ExternalInput")
    out_d = nc.dram_tensor("out", [2, NPTS // 2], f32, kind="ExternalOutput")
    cc_in = nc.dram_tensor("cc_in", [128, 1], f32)
    cc_out = nc.dram_tensor("cc_out", [128, 1], f32, addr_space="Shared")

    def TT(eng, out, i0, i1, op=ALU.mult):
        eng.tensor_tensor(out, i0, i1, op)

    with tile.TileContext(nc) as tc:
        with (
            tc.tile_pool(name="smp", bufs=1) as smp,
            tc.tile_pool(name="persist", bufs=1) as persist,
            tc.tile_pool(name="wpool", bufs=4) as wpool,
            tc.tile_pool(name="scr", bufs=1) as scr,
            tc.tile_pool(name="ps8", bufs=1, space="PSUM") as ps,
        ):
            # ---- packed constant loads (6 DMA issues) ----
            x4 = smp.tile([4, NPTS], f16, name="x4t")
            nc.sync.dma_start(x4[:], x_d.ap())
            w11 = smp.tile([4, 128], f16, name="w11t")
            nc.sync.dma_start(w11[:], w11_d.ap())
            pk32 = smp.tile([128, PK32_COLS], f32, name="pk32t")
            nc.sync.dma_start(pk32[:], pk32_d.ap())
            a2 = smp.tile([128, NKT, 2], f16, name="a2t")
            nc.sync.dma_start(a2[:], a_d.ap())
            pkb = smp.tile([128, PKB_COLS], f16, name="pkbt")
            nc.sync.dma_start(pkb[:], pkb_d.ap())
            pkc = smp.tile([128, PKC_COLS], f16, name="pkct")
            nc.sync.dma_start(pkc[:], pkc_d.ap())

            smt = {"w11": w11[:]}
            for n_, c0 in _PKB.items():
                smt[n_] = pkb[:, c0:c0 + 128]
            for n_, c0 in _PKC.items():
                if n_ == "v6":
                    smt[n_] = pkc[:, c0:c0 + 6]
                elif n_ == "pq2":
                    smt[n_] = pkc[0:1, c0:c0 + 128]
                else:
                    smt[n_] = pkc[:, c0:c0 + 128]
            for n_, c0 in _PK32.items():
                if n_ == "sel4m":
                    smt[n_] = pk32[0:8, c0:c0 + 4]
                else:
                    smt[n_] = pk32[:, c0:c0 + 1]

            # ---- trunk layer-1 z1 matmuls (pairs f, f+4) ----
            zpair = [("zA", "zB"), ("aA", "aB")]
            z1ps = {}
            for j in range(NEN):
                tA, tB = zpair[j % 2]
                for hx, f in enumerate((j, j + 4)):
                    cs = slice(f * FD, (f + 1) * FD)
                    z1 = ps.tile([128, FD], f32, tag=(tA, tB)[hx], name=f"z1_{f}")
                    nc.tensor.matmul(z1[:], smt["w11"], x4[:, cs], start=True,
                                     stop=True)
                    z1ps[f] = z1

            # ---- trunk layer-1 elementwise (wide pairs, single-fp16 t1/tp1) --
            l1 = {}
            for j in range(NEN):
                t1f = scr.tile([128, WFD], f32, tag="t1f", name=f"t1f_{j}")
                for hx, f in enumerate((j, j + 4)):
                    hs = slice(hx * FD, (hx + 1) * FD)
                    nc.scalar.activation(t1f[:, hs], z1ps[f][:], AF.Tanh,
                                         bias=smt["c1b"])
                t1h = persist.tile([128, WFD], f16, tag=f"t1h_{j % 2}", name=f"t1h_{j}")
                nc.scalar.copy(t1h[:], t1f[:])
                s1 = scr.tile([128, WFD], f32, tag="s1", name=f"s1_{j}")
                nc.scalar.square(s1[:], t1f[:])
                tp1f = scr.tile([128, WFD], f32, tag="tp1f", name=f"tp1f_{j}")
                nc.scalar.activation(tp1f[:], s1[:], AF.Copy, bias=1.0, scale=-1.0)
                tp1h = persist.tile([128, WFD], f16, tag=f"tp1h_{j % 2}", name=f"tp1h_{j}")
                nc.scalar.copy(tp1h[:], tp1f[:])
                g2m = persist.tile([128, WFD], f16, tag=f"g2m_{j % 2}", name=f"g2m_{j}")
                TT(nc.vector, g2m[:], t1h[:], tp1h[:])
                # g3m = (tp1-2/3)*tp1 = -(s1-1/3)*tp1; sign folded into H below
                g3m = persist.tile([128, WFD], f16, tag=f"g3m_{j % 2}", name=f"g3m_{j}")
                nc.vector.scalar_tensor_tensor(
                    g3m[:], tp1h[:], 2.0 / 3.0, tp1h[:], ALU.subtract, ALU.mult)
                l1[j] = (t1h, tp1h, g2m, g3m)

            # ---- matvec: stream W shard (2 DMA splits per 1MB chunk) ----
            b8 = ps.tile([8, FD], f32, tag="pMV", name="b8")
            half = KTC * 64
            for i in range(NCHUNK):
                wch = wpool.tile([128, KTC * 128], f16, tag="wch", name="wch")
                nc.sync.dma_start(wch[:, 0:half], w_d.ap()[i][:, 0:half])
                nc.sync.dma_start(wch[:, half:], w_d.ap()[i][:, half:])
                for g in range(KTC // 4):
                    nc.tensor.matmul(
                        b8[:], a2[:, i * KTC + 4 * g:i * KTC + 4 * (g + 1), :],
                        wch[:, g * 512:(g + 1) * 512],
                        start=(i == 0 and g == 0),
                        stop=(i == NCHUNK - 1 and g == KTC // 4 - 1),
                    )

            # ---- local reduce + AllReduce ----
            b8sb = smp.tile([8, FD], f32, name="b8sb")
            nc.scalar.copy(b8sb[:], b8[:])
            bcol = ps.tile([128, 1], f32, tag="pBC", name="bcol")
            for j in range(4):
                nc.tensor.matmul(bcol[:], b8sb[:, j * 128:(j + 1) * 128],
                                 smt["sel4m"][:, j:j + 1],
                                 start=(j == 0), stop=(j == 3))
            b_loc = smp.tile([128, 1], f32, name="bloc")
            nc.scalar.copy(b_loc[:], bcol[:])
            nc.sync.dma_start(cc_in.ap(), b_loc[:])
            nc.gpsimd.collective_compute(
                "AllReduce", ALU.add,
                replica_groups=[list(range(NCORES))],
                ins=[cc_in.ap()], outs=[cc_out.ap()],
            )
            b_ar = smp.tile([128, 1], f32, name="bar")
            nc.sync.dma_start(b_ar[:], cc_out.ap())

            # ---- trunk layer-2 wave (wide pairs) ----
            sh = {}
            for j in range(NEN):
                t1h, tp1h, g2m, g3m = l1[j]
                t2f_s = persist.tile([128, WFD], f32, tag=f"sht2_{j}",
                                     name=f"sh_t2_{j}")
                P1f_s = persist.tile([128, WFD], f32, tag=f"shP1_{j}",
                                     name=f"sh_P1_{j}")
                ux2_s = persist.tile([128, WFD], f16, tag=f"shux2_{j}",
                                     name=f"sh_ux2_{j}")
                ux3_s = persist.tile([128, WFD], f16, tag=f"shux3_{j}",
                                     name=f"sh_ux3_{j}")
                sh[j] = (t2f_s, P1f_s, ux2_s, ux3_s)
                ge = nc.gpsimd if j < 2 else nc.vector

                zw, aw = [], []
                for hx in range(2):
                    hs = slice(hx * FD, (hx + 1) * FD)
                    z2 = ps.tile([128, FD], f32, tag=zpair[0][hx], name=f"z2_{j}{hx}")
                    nc.tensor.matmul(z2[:], smt["wt2h"], t1h[:, hs], start=True,
                                     stop=False)
                    nc.tensor.matmul(z2[:], smt["wt2l"], t1h[:, hs], start=False,
                                     stop=True)
                    zw.append(z2)
                    A = ps.tile([128, FD], f32, tag=zpair[1][hx], name=f"A_{j}{hx}")
                    nc.tensor.matmul(A[:], smt["w2ah"], tp1h[:, hs], start=True,
                                     stop=False)
                    nc.tensor.matmul(A[:], smt["w2al"], tp1h[:, hs], start=False,
                                     stop=True)
                    aw.append(A)

                Bc = scr.tile([128, WFD], f16, tag="Bc", name=f"Bc_{j}")
                Cc = scr.tile([128, WFD], f16, tag="Cc", name=f"Cc_{j}")
                A2c = scr.tile([128, WFD], f16, tag="A2c", name=f"A2c_{j}")
                Acp = scr.tile([128, WFD], f16, tag="Acp", name=f"Acp_{j}")
                for hx in range(2):
                    hs = slice(hx * FD, (hx + 1) * FD)
                    nc.scalar.activation(t2f_s[:, hs], zw[hx][:], AF.Tanh,
                                         bias=smt["bt2b"])
                    nc.scalar.square(A2c[:, hs], aw[hx][:])
                    nc.scalar.copy(Acp[:, hs], aw[hx][:])
                    B = ps.tile([128, FD], f32, tag="pB", name=f"B_{j}{hx}")
                    nc.tensor.matmul(B[:], smt["w2b"], g2m[:, hs], start=True,
                                     stop=True)
                    nc.scalar.copy(Bc[:, hs], B[:])
                    C = ps.tile([128, FD], f32, tag="pC", name=f"C_{j}{hx}")
                    nc.tensor.matmul(C[:], smt["w2c"], g3m[:, hs], start=True,
                                     stop=True)
                    nc.scalar.copy(Cc[:, hs], C[:])

                t2c = scr.tile([128, WFD], f16, tag="t2c", name=f"t2c_{j}")
                nc.scalar.copy(t2c[:], t2f_s[:])
                s2 = scr.tile([128, WFD], f32, tag="s2", name=f"s2_{j}")
                nc.scalar.square(s2[:], t2f_s[:])
                tp2 = scr.tile([128, WFD], f32, tag="tp2", name=f"tp2_{j}")
                nc.vector.tensor_scalar(tp2[:], s2[:], -1.0, 1.0, ALU.mult, ALU.add)
                tp2c = scr.tile([128, WFD], f16, tag="tp2c", name=f"tp2c_{j}")
                nc.scalar.activation(tp2c[:], s2[:], AF.Copy, bias=1.0, scale=-1.0)
                for hx in range(2):
                    hs = slice(hx * FD, (hx + 1) * FD)
                    TT(nc.vector, P1f_s[:, hs], tp2[:, hs], aw[hx][:])

                T1 = scr.tile([128, WFD], f16, tag="T1", name=f"T1_{j}")
                TT(ge, T1[:], t2c[:], A2c[:])
                E = scr.tile([128, WFD], f16, tag="E", name=f"E_{j}")
                nc.vector.scalar_tensor_tensor(
                    E[:], T1[:], -2.0, Bc[:], ALU.mult, ALU.add)
                TT(nc.vector, ux2_s[:], tp2c[:], E[:])
                A3 = scr.tile([128, WFD], f16, tag="A3", name=f"A3_{j}")
                TT(ge, A3[:], A2c[:], Acp[:])
                G1 = scr.tile([128, WFD], f16, tag="G1", name=f"G1_{j}")
                nc.vector.scalar_tensor_tensor(
                    G1[:], tp2c[:], 2.0 / 3.0, A3[:], ALU.subtract, ALU.mult)
                G2 = scr.tile([128, WFD], f16, tag="G2", name=f"G2_{j}")
                TT(ge, G2[:], t2c[:], Acp[:])
                G3 = scr.tile([128, WFD], f16, tag="G3", name=f"G3_{j}")
                TT(ge, G3[:], G2[:], Bc[:])
                D = scr.tile([128, WFD], f16, tag="Dd", name=f"D_{j}")
                TT(nc.vector, D[:], G1[:], G3[:], ALU.add)
                # Cc holds -C_true (g3m sign-flip): H = -6*D - Cc = -6*D + C
                H = scr.tile([128, WFD], f16, tag="Hh", name=f"H_{j}")
                nc.vector.scalar_tensor_tensor(
                    H[:], D[:], -6.0, Cc[:], ALU.mult, ALU.subtract)
                TT(nc.vector, ux3_s[:], tp2c[:], H[:])

            # ---- b -> c -> outer-product stationaries S = c(x)p, c(x)q ----
            b16 = smp.tile([128, 1], f16, name="b16")
            nc.scalar.copy(b16[:], b_ar[:])
            c0p = ps.tile([1, 128], f32, tag="pBC", name="c0p")
            nc.tensor.matmul(c0p[:], b16[:], smt["wt3h"], start=True, stop=False)
            nc.tensor.matmul(c0p[:], b16[:], smt["wt3l"], start=False, stop=True)
            c0sb = smp.tile([1, 128], f16, name="c0sb")
            nc.scalar.copy(c0sb[:], c0p[:])
            scpq_p = ps.tile([128, 128], f32, tag="pBC", name="scpq_p")
            nc.tensor.matmul(scpq_p[:], c0sb[:], smt["pq2"], start=True, stop=True)
            Scpq32 = smp.tile([128, 128], f32, name="Scpq32")
            nc.scalar.copy(Scpq32[:], scpq_p[:])
            Scpq16 = smp.tile([128, 128], f16, name="Scpq16")
            nc.scalar.copy(Scpq16[:], scpq_p[:])
            Sp32, Sq32 = Scpq32[:, 0:64], Scpq32[:, 64:128]
            Sp16, Sq16 = Scpq16[:, 0:64], Scpq16[:, 64:128]

            # ---- energy phase ----
            for e in range(NEN):
                t2f_s, P1f_s, ux2_s, ux3_s = sh[e]
                trio = [["zA", "zB", "aA"], ["aB", "pB", "pC"]][e % 2]
                dzt, dyt = ("pBC", "pMV") if e % 2 == 0 else ("pMV", "pBC")

                z1e = ps.tile([128, FD], f32, tag=trio[0], name=f"z1e_{e}")
                z1p = ps.tile([128, FD], f32, tag=trio[1], name=f"z1p_{e}")
                z1pp = ps.tile([128, FD], f32, tag=trio[2], name=f"z1pp_{e}")
                for hx in range(2):
                    hs = slice(hx * FD, (hx + 1) * FD)
                    rs = slice(hx * 64, (hx + 1) * 64)
                    nc.tensor.matmul(z1e[rs, :], Sp32, t2f_s[:, hs], start=True,
                                     stop=False)
                    nc.tensor.matmul(z1e[rs, :], Sq32, P1f_s[:, hs], start=False,
                                     stop=True)
                    nc.tensor.matmul(z1p[rs, :], Sp32, P1f_s[:, hs], start=True,
                                     stop=False)
                    nc.tensor.matmul(z1p[rs, :], Sq16, ux2_s[:, hs], start=False,
                                     stop=True)
                    nc.tensor.matmul(z1pp[rs, :], Sp16, ux2_s[:, hs], start=True,
                                     stop=False)
                    nc.tensor.matmul(z1pp[rs, :], Sq16, ux3_s[:, hs], start=False,
                                     stop=True)

                t1ef = scr.tile([128, FD], f32, tag="t1ef", name=f"t1ef_{e}")
                nc.scalar.activation(t1ef[:], z1e[:], AF.Tanh, bias=smt["be1b2"])
                t1eh = scr.tile([128, FD], f16, tag="t1eh", name=f"t1eh_{e}")
                nc.scalar.copy(t1eh[:], t1ef[:])
                z1psb = scr.tile([128, FD], f16, tag="z1psb", name=f"z1psb_{e}")
                nc.scalar.copy(z1psb[:], z1p[:])
                z1ppsb = scr.tile([128, FD], f16, tag="z1ppsb", name=f"z1ppsb_{e}")
                nc.scalar.copy(z1ppsb[:], z1pp[:])
                s1e = scr.tile([128, FD], f16, tag="s1e", name=f"s1e_{e}")
                nc.scalar.square(s1e[:], t1ef[:])
                m_ = scr.tile([128, FD], f16, tag="m_", name=f"m_{e}")
                nc.scalar.activation(m_[:], s1e[:], AF.Copy, bias=1.0, scale=-1.0)
                z1p2 = scr.tile([128, FD], f16, tag="z1p2", name=f"z1p2_{e}")
                TT(nc.gpsimd, z1p2[:], z1psb[:], z1psb[:])
                N1 = scr.tile([128, FD], f16, tag="N1", name=f"N1_{e}")
                TT(nc.vector, N1[:], t1eh[:], m_[:])
                a1p = scr.tile([128, FD], f16, tag="a1p", name=f"a1p_{e}")
                TT(nc.vector, a1p[:], m_[:], z1psb[:])
                N2 = scr.tile([128, FD], f16, tag="N2", name=f"N2_{e}")
                TT(nc.gpsimd, N2[:], N1[:], z1p2[:])
                N3 = scr.tile([128, FD], f16, tag="N3", name=f"N3_{e}")
                TT(nc.vector, N3[:], m_[:], z1ppsb[:])
                zin = scr.tile([128, FD], f16, tag="zin", name=f"zin_{e}")
                nc.vector.scalar_tensor_tensor(
                    zin[:], N2[:], -2.0, N3[:], ALU.mult, ALU.add)
                mpc = scr.tile([128, FD], f16, tag="mpc", name=f"mpc_{e}")
                TT(nc.vector, mpc[:], N1[:], z1psb[:])
                O1 = scr.tile([128, FD], f16, tag="O1", name=f"O1_{e}")
                nc.vector.scalar_tensor_tensor(
                    O1[:], s1e[:], 1.0 / 3.0, m_[:], ALU.subtract, ALU.mult)
                O2f = scr.tile([128, FD], f16, tag="O2f", name=f"O2f_{e}")
                TT(nc.gpsimd, O2f[:], O1[:], z1p2[:])
                O3f = scr.tile([128, FD], f16, tag="O3f", name=f"O3f_{e}")
                TT(nc.vector, O3f[:], N1[:], z1ppsb[:])
                O2m = scr.tile([128, FD], f16, tag="O2m", name=f"O2m_{e}")
                nc.vector.scalar_tensor_tensor(
                    O2m[:], O2f[:], 3.0, O3f[:], ALU.mult, ALU.subtract)

                z2e = ps.tile([128, FD], f32, tag=trio[0], name=f"z2e_{e}")
                nc.tensor.matmul(z2e[:], smt["e0"], t1eh[:], start=True, stop=True)
                z2ep = ps.tile([128, FD], f32, tag=trio[1], name=f"z2ep_{e}")
                nc.tensor.matmul(z2ep[:], smt["e0"], a1p[:], start=True, stop=True)
                z2epp = ps.tile([128, FD], f32, tag=trio[2], name=f"z2epp_{e}")
                nc.tensor.matmul(z2epp[:], smt["e0"], zin[:], start=True, stop=True)
                Dz = ps.tile([128, FD], f32, tag=dzt, name=f"Dz_{e}")
                nc.tensor.matmul(Dz[:], smt["eq"], m_[:], start=True, stop=True)
                DyN = ps.tile([128, FD], f32, tag=dyt, name=f"DyN_{e}")
                nc.tensor.matmul(DyN[:], smt["ep"], m_[:], start=True, stop=True)
                DzpN = ps.tile([128, FD], f32, tag=trio[0], name=f"DzpN_{e}")
                nc.tensor.matmul(DzpN[:], smt["eq"], mpc[:], start=True, stop=True)
                DypN = ps.tile([128, FD], f32, tag=trio[1], name=f"DypN_{e}")
                nc.tensor.matmul(DypN[:], smt["ep"], mpc[:], start=True, stop=True)
                Dzpp2 = ps.tile([128, FD], f32, tag=trio[2], name=f"Dzpp2_{e}")
                nc.tensor.matmul(Dzpp2[:], smt["eq"], O2m[:], start=True, stop=True)

                t2e = scr.tile([128, FD], f16, tag="t2e", name=f"t2e_{e}")
                nc.scalar.activation(t2e[:], z2e[:], AF.Tanh, bias=smt["be2b2"])
                s2e = scr.tile([128, FD], f16, tag="s2e", name=f"s2e_{e}")
                nc.scalar.square(s2e[:], t2e[:])
                w_ = scr.tile([128, FD], f16, tag="w_", name=f"w_{e}")
                nc.scalar.activation(w_[:], s2e[:], AF.Copy, bias=1.0, scale=-1.0)
                z2ep16 = scr.tile([128, FD], f16, tag="z2ep16", name=f"z2ep16_{e}")
                nc.scalar.copy(z2ep16[:], z2ep[:])
                z2ep2 = scr.tile([128, FD], f16, tag="z2ep2", name=f"z2ep2_{e}")
                TT(nc.gpsimd, z2ep2[:], z2ep16[:], z2ep16[:])
                Q1 = scr.tile([128, FD], f16, tag="Q1", name=f"Q1_{e}")
                TT(nc.vector, Q1[:], t2e[:], w_[:])
                wpc = scr.tile([128, FD], f16, tag="wpc", name=f"wpc_{e}")
                TT(nc.vector, wpc[:], Q1[:], z2ep16[:])
                R1 = scr.tile([128, FD], f16, tag="R1", name=f"R1_{e}")
                nc.vector.scalar_tensor_tensor(
                    R1[:], s2e[:], 1.0 / 3.0, w_[:], ALU.subtract, ALU.mult)
                R2f = scr.tile([128, FD], f16, tag="R2f", name=f"R2f_{e}")
                TT(nc.gpsimd, R2f[:], R1[:], z2ep2[:])
                R3f = scr.tile([128, FD], f16, tag="R3f", name=f"R3f_{e}")
                TT(nc.vector, R3f[:], Q1[:], z2epp[:])
                t1m = scr.tile([128, FD], f16, tag="t1m", name=f"t1m_{e}")
                nc.vector.scalar_tensor_tensor(
                    t1m[:], R2f[:], 3.0, R3f[:], ALU.mult, ALU.subtract)
                F1 = scr.tile([128, FD], f16, tag="F1", name=f"F1_{e}")
                TT(nc.vector, F1[:], t1m[:], Dz[:])
                DyNs = scr.tile([128, FD], f16, tag="DyNs", name=f"DyNs_{e}")
                nc.scalar.copy(DyNs[:], DyN[:])
                t2m = scr.tile([128, FD], f16, tag="t2m", name=f"t2m_{e}")
                nc.vector.scalar_tensor_tensor(
                    t2m[:], DzpN[:], 4.0, DyNs[:], ALU.mult, ALU.add)
                F2 = scr.tile([128, FD], f16, tag="F2", name=f"F2_{e}")
                TT(nc.gpsimd, F2[:], wpc[:], t2m[:])
                DypNs = scr.tile([128, FD], f16, tag="DypNs", name=f"DypNs_{e}")
                nc.scalar.copy(DypNs[:], DypN[:])
                t3m = scr.tile([128, FD], f16, tag="t3m", name=f"t3m_{e}")
                TT(nc.vector, t3m[:], Dzpp2[:], DypNs[:], ALU.add)
                F3 = scr.tile([128, FD], f16, tag="F3", name=f"F3_{e}")
                TT(nc.vector, F3[:], w_[:], t3m[:])

                vps = ps.tile([2, FD], f32, tag=trio[1], name=f"vps_{e}")
                nc.tensor.matmul(vps[:], smt["v6"][:, 0:2], F1[:], start=True,
                                 stop=False)
                nc.tensor.matmul(vps[:], smt["v6"][:, 2:4], F2[:], start=False,
                                 stop=False)
                nc.tensor.matmul(vps[:], smt["v6"][:, 4:6], F3[:], start=False,
                                 stop=True)
                ot = scr.tile([2, FD], f32, tag="ot", name=f"ot_{e}")
                nc.scalar.copy(ot[:], vps[:])
                nc.sync.dma_start(out_d.ap()[:, e * FD:(e + 1) * FD], ot[:])

    nc.compile()
    return nc


def _get_nc():
    if "nc" not in _CACHE:
        _CACHE["nc"] = _build()
    return _CACHE["nc"]


def kernel(**inputs):
    import concourse.bass_utils as bass_utils

    f = lambda k: np.asarray(inputs[k], np.float32)
    a, x, t = f("a"), f("x"), np.float32(inputs["t"])
    Wb, Wt1, bt1, Wt2, bt2 = f("Wb"), f("Wt1"), f("bt1"), f("Wt2"), f("bt2")
    Wt3, We1, be1, We2, be2, We3 = (
        f("Wt3"), f("We1"), f("be1"), f("We2"), f("be2"), f("We3"))

    h16 = lambda v: np.asarray(v, np.float32).astype(np.float16)
    def pair16(v):
        h = h16(v)
        return h, h16(np.asarray(v, np.float32) - h.astype(np.float32))

    w1 = Wt1[:, 0]
    c1b = (Wt1[:, 1] * t + bt1)[:, None]
    w1h, w1l = pair16(w1)
    w11 = np.stack([w1h, w1h, w1l, w1l])                       # [4,128]
    wt2t = np.ascontiguousarray(Wt2.T)
    wt2h, wt2l = pair16(wt2t)
    w2ah, w2al = pair16(wt2t * w1[:, None])
    w2b = h16(wt2t * (-2.0 * w1 ** 2)[:, None])
    w2c = h16(wt2t * (6.0 * w1 ** 3)[:, None])
    wt3h, wt3l = pair16(Wt3)

    p, q, v = We1[:, 0], We1[:, 1], We3[0]
    pq2 = np.zeros((1, 128), np.float16)
    pq2[0, 0:64] = h16(p)
    pq2[0, 64:128] = h16(q)

    blk = lambda M: np.block([[M, np.zeros_like(M)], [np.zeros_like(M), M]])
    We2T = We2.T
    e0 = h16(blk(We2T))
    eq = h16(blk(We2T * q[:, None]))
    ep = h16(blk(We2T * p[:, None]))
    v6 = np.zeros((128, 6), np.float16)
    for i in range(3):
        v6[0:64, 2 * i] = h16(2.0 * v)
        v6[64:128, 2 * i + 1] = h16(2.0 * v)
    sel4m = np.zeros((8, 4), np.float32)
    for j in range(4):
        sel4m[2 * j, j] = 1.0
        sel4m[2 * j + 1, j] = 1.0

    pkb = np.zeros((128, PKB_COLS), np.float16)
    for n_, arr in [("wt2h", wt2h), ("wt2l", wt2l), ("w2ah", w2ah),
                    ("w2al", w2al), ("w2b", w2b), ("w2c", w2c)]:
        pkb[:, _PKB[n_]:_PKB[n_] + 128] = arr
    pkc = np.zeros((128, PKC_COLS), np.float16)
    for n_, arr in [("wt3h", wt3h), ("wt3l", wt3l), ("e0", e0), ("eq", eq),
                    ("ep", ep)]:
        pkc[:, _PKC[n_]:_PKC[n_] + 128] = arr
    pkc[0:1, _PKC["pq2"]:_PKC["pq2"] + 128] = pq2
    pkc[:, _PKC["v6"]:_PKC["v6"] + 6] = v6
    pk32 = np.zeros((128, PK32_COLS), np.float32)
    pk32[:, 0] = c1b[:, 0]
    pk32[:, 1] = bt2
    pk32[:, 2] = np.concatenate([be1, be1])
    pk32[:, 3] = np.concatenate([be2, be2])
    pk32[0:8, 4:8] = sel4m

    smalls = {
        "w11": np.ascontiguousarray(w11),
        "pkb": np.ascontiguousarray(pkb),
        "pkc": np.ascontiguousarray(pkc),
        "pk32": np.ascontiguousarray(pk32),
    }

    in_maps = []
    for c in range(NCORES):
        blk_w = Wb[:, c * KSH:(c + 1) * KSH]                   # [128, 65536]
        tr = blk_w.T.reshape(NKT, 128, 128).transpose(1, 0, 2)  # [k1, kt, p]
        tr = tr.reshape(128, NCHUNK, KTC * 128).transpose(1, 0, 2)
        wsh = np.ascontiguousarray(h16(1024.0 * tr))           # [16,128,4096]
        ash = (a[c * KSH:(c + 1) * KSH] / 1024.0).reshape(NKT, 128).T  # [k1, kt]
        ah, al = pair16(ash)
        a2 = np.ascontiguousarray(np.stack([ah, al], axis=2))  # [128,512,2]
        xs = x[c * NPTS:(c + 1) * NPTS]
        xh, xl = pair16(xs)
        x4 = np.ascontiguousarray(np.stack([xh, xl, xh, xl]))  # [4,4096]
        im = {"w": wsh, "a2": a2, "x4": x4}
        im.update(smalls)
        in_maps.append(im)

    global _last_in_maps
    _last_in_maps = in_maps
    nc = _get_nc()
    res = bass_utils.run_bass_kernel_spmd(nc, in_maps, core_ids=list(range(NCORES)))
    outs = []
    for c in range(NCORES):
        o = res.results[c]["out"]          # [2, NPTS//2]
        outs.append(np.asarray(o).reshape(-1))
    return np.concatenate(outs).astype(np.float32)


# revision 13
# speedup vs baseline: 1.3566x; 1.0354x over previous
"""Bass/Trainium2 kernel for nn_HNO_37065567764989 (self-contained).

Strategy (8 NeuronCores, SPMD):
- Branch matvec b = Wb@a column-sharded 8 ways. Each core streams its 16MB
  shard as fp16 (W scaled by 2^10; a as an fp16 hi/lo stationary pair), two
  512KB DMAs per 1MB chunk across queues. 512B AllReduce combines partials.
- Nx=32768 points sharded 8 ways (4096/core). Trunk runs as 4 wide pairs
  (tiles f and f+4 share [128,1024] elementwise ops that write the energy
  movings directly). GpSimd carries only early-pair products plus the
  collective, so the mesh wait never blocks the trunk tail.
- EnergyNet first layer uses runtime outer-product stationaries S=c(x)p,
  c(x)q built on-device after the AllReduce -- no per-row extraction.
- Precision: t2/P1 flow as fp16 hi/lo pairs; t1/tp1, B/C stationaries and
  all product chains are single fp16 (mirror-validated 1.24e-2).
"""
import sys

for _p in ("/opt/trn_rl_repo",):
    if _p not in sys.path:
        sys.path.insert(0, _p)

import numpy as np

MP1, NX, P, HT, HE = 524288, 32768, 128, 128, 64
NCORES = 8
KSH = MP1 // NCORES        # 65536 contraction elems per core
NKT = KSH // 128           # 512 k-tiles
NCHUNK = 16
KTC = NKT // NCHUNK        # 32 k-tiles per chunk
NPTS = NX // NCORES        # 4096 points per core
FD = 512
WFD = 2 * FD               # wide pair width
NTRUNK = NPTS // FD        # 8 trunk tiles
NEN = NTRUNK // 2          # 4 energy tiles / trunk pairs

_PKB = {"wt2h": 0, "wt2l": 128, "w2ah": 256, "w2al": 384, "w2b": 512,
        "w2c": 640}
PKB_COLS = 768
_PKC = {"wt3h": 0, "wt3l": 128, "e0": 256, "eq": 384, "ep": 512,
        "pq2": 640, "v6": 768}
PKC_COLS = 774
_PK32 = {"c1b": 0, "bt2b": 1, "be1b2": 2, "be2b2": 3, "sel4m": 4}
PK32_COLS = 8

_CACHE = {}


def _build():
    import concourse.bacc as bacc
    import concourse.mybir as mybir
    from concourse import tile

    f32 = mybir.dt.float32
    f16 = mybir.dt.float16
    AF = mybir.ActivationFunctionType
    ALU = mybir.AluOpType

    nc = bacc.Bacc("TRN2", target_bir_lowering=False, debug=False,
                   num_devices=NCORES)

    w_d = nc.dram_tensor("w", [NCHUNK, 128, KTC * 128], f16, kind="ExternalInput")
    a_d = nc.dram_tensor("a2", [128, NKT, 2], f16, kind="ExternalInput")
    x_d = nc.dram_tensor("x4", [4, NPTS], f16, kind="ExternalInput")
    w11_d = nc.dram_tensor("w11", [4, 128], f16, kind="ExternalInput")
    pkb_d = nc.dram_tensor("pkb", [128, PKB_COLS], f16, kind="ExternalInput")
    pkc_d = nc.dram_tensor("pkc", [128, PKC_COLS], f16, kind="ExternalInput")
    pk32_d = nc.dram_tensor("pk32", [128, PK32_COLS], f32, kind="ExternalInput")
    out_d = nc.dram_tensor("out", [2, NPTS // 2], f32, kind="ExternalOutput")
    cc_in = nc.dram_tensor("cc_in", [128, 1], f32)
    cc_out = nc.dram_tensor("cc_out", [128, 1], f32, addr_space="Shared")

    def TT(eng, out, i0, i1, op=ALU.mult):
        eng.tensor_tensor(out, i0, i1, op)

    with tile.TileContext(nc) as tc:
        with (
            tc.tile_pool(name="smp", bufs=1) as smp,
            tc.tile_pool(name="persist", bufs=1) as persist,
            tc.tile_pool(name="wpool", bufs=4) as wpool,
            tc.tile_pool(name="scr", bufs=1) as scr,
            tc.tile_pool(name="ps8", bufs=1, space="PSUM") as ps,
        ):
            # ---- packed constant loads (6 DMA issues) ----
            x4 = smp.tile([4, NPTS], f16, name="x4t")
            nc.sync.dma_start(x4[:], x_d.ap())
            w11 = smp.tile([4, 128], f16, name="w11t")
            nc.sync.dma_start(w11[:], w11_d.ap())
            pk32 = smp.tile([128, PK32_COLS], f32, name="pk32t")
            nc.sync.dma_start(pk32[:], pk32_d.ap())
            a2 = smp.tile([128, NKT, 2], f16, name="a2t")
            nc.sync.dma_start(a2[:], a_d.ap())
            pkb = smp.tile([128, PKB_COLS], f16, name="pkbt")
            nc.sync.dma_start(pkb[:], pkb_d.ap())
            pkc = smp.tile([128, PKC_COLS], f16, name="pkct")
            nc.sync.dma_start(pkc[:], pkc_d.ap())

            smt = {"w11": w11[:]}
            for n_, c0 in _PKB.items():
                smt[n_] = pkb[:, c0:c0 + 128]
            for n_, c0 in _PKC.items():
                if n_ == "v6":
                    smt[n_] = pkc[:, c0:c0 + 6]
                elif n_ == "pq2":
                    smt[n_] = pkc[0:1, c0:c0 + 128]
                else:
                    smt[n_] = pkc[:, c0:c0 + 128]
            for n_, c0 in _PK32.items():
                if n_ == "sel4m":
                    smt[n_] = pk32[0:8, c0:c0 + 4]
                else:
                    smt[n_] = pk32[:, c0:c0 + 1]

            # ---- trunk layer-1 z1 matmuls (pairs f, f+4) ----
            # z1 borrows pB/pC/pBC banks so zA/zB/aA/aB stay free for the
            # layer-2 wave to start as soon as l1 of pair 0 is done.
            zpair = [("zA", "zB"), ("aA", "aB")]
            z1tags = ["pB", "pC", "pBC"]
            z1ps = {}
            zi = 0
            for j in range(NEN):
                for hx, f in enumerate((j, j + 4)):
                    cs = slice(f * FD, (f + 1) * FD)
                    z1 = ps.tile([128, FD], f32, tag=z1tags[zi % 3], name=f"z1_{f}")
                    zi += 1
                    nc.tensor.matmul(z1[:], smt["w11"], x4[:, cs], start=True,
                                     stop=True)
                    z1ps[f] = z1

            # ---- trunk layer-1 elementwise (wide pairs, single-fp16 t1/tp1) --
            l1 = {}
            for j in range(NEN):
                t1f = scr.tile([128, WFD], f32, tag="t1f", name=f"t1f_{j}")
                for hx, f in enumerate((j, j + 4)):
                    hs = slice(hx * FD, (hx + 1) * FD)
                    nc.scalar.activation(t1f[:, hs], z1ps[f][:], AF.Tanh,
                                         bias=smt["c1b"])
                t1h = persist.tile([128, WFD], f16, tag=f"t1h_{j % 2}", name=f"t1h_{j}")
                nc.scalar.copy(t1h[:], t1f[:])
                s1 = scr.tile([128, WFD], f32, tag="s1", name=f"s1_{j}")
                nc.scalar.square(s1[:], t1f[:])
                tp1f = scr.tile([128, WFD], f32, tag="tp1f", name=f"tp1f_{j}")
                nc.scalar.activation(tp1f[:], s1[:], AF.Copy, bias=1.0, scale=-1.0)
                tp1h = persist.tile([128, WFD], f16, tag=f"tp1h_{j % 2}", name=f"tp1h_{j}")
                nc.scalar.copy(tp1h[:], tp1f[:])
                g2m = persist.tile([128, WFD], f16, tag=f"g2m_{j % 2}", name=f"g2m_{j}")
                TT(nc.vector, g2m[:], t1h[:], tp1h[:])
                # g3m = (tp1-2/3)*tp1 = -(s1-1/3)*tp1; sign folded into H below
                g3m = persist.tile([128, WFD], f16, tag=f"g3m_{j % 2}", name=f"g3m_{j}")
                nc.vector.scalar_tensor_tensor(
                    g3m[:], tp1h[:], 2.0 / 3.0, tp1h[:], ALU.subtract, ALU.mult)
                l1[j] = (t1h, tp1h, g2m, g3m)

            # ---- matvec: stream W shard (2 DMA splits per 1MB chunk) ----
            b8 = ps.tile([8, FD], f32, tag="pMV", name="b8")
            half = KTC * 64
            for i in range(NCHUNK):
                wch = wpool.tile([128, KTC * 128], f16, tag="wch", name="wch")
                nc.sync.dma_start(wch[:, 0:half], w_d.ap()[i][:, 0:half])
                nc.sync.dma_start(wch[:, half:], w_d.ap()[i][:, half:])
                for g in range(KTC // 4):
                    nc.tensor.matmul(
                        b8[:], a2[:, i * KTC + 4 * g:i * KTC + 4 * (g + 1), :],
                        wch[:, g * 512:(g + 1) * 512],
                        start=(i == 0 and g == 0),
                        stop=(i == NCHUNK - 1 and g == KTC // 4 - 1),
                    )

            # ---- local reduce + AllReduce (high priority: the mesh gates
            # the whole energy phase, so these must not sit behind the
            # trunk wave in the engine queues) ----
            with tc.high_priority():
                b8sb = smp.tile([8, FD], f32, name="b8sb")
                nc.scalar.copy(b8sb[:], b8[:])
                bcol = ps.tile([128, 1], f32, tag="pBC", name="bcol")
                for j in range(4):
                    nc.tensor.matmul(bcol[:], b8sb[:, j * 128:(j + 1) * 128],
                                     smt["sel4m"][:, j:j + 1],
                                     start=(j == 0), stop=(j == 3))
                b_loc = smp.tile([128, 1], f32, name="bloc")
                nc.scalar.copy(b_loc[:], bcol[:])
                nc.sync.dma_start(cc_in.ap(), b_loc[:])
                nc.gpsimd.collective_compute(
                    "AllReduce", ALU.add,
                    replica_groups=[list(range(NCORES))],
                    ins=[cc_in.ap()], outs=[cc_out.ap()],
                )
                b_ar = smp.tile([128, 1], f32, name="bar")
                nc.sync.dma_start(b_ar[:], cc_out.ap())

            # ---- trunk layer-2 wave (wide pairs) ----
            sh = {}
            for j in range(NEN):
                t1h, tp1h, g2m, g3m = l1[j]
                t2f_s = persist.tile([128, WFD], f32, tag=f"sht2_{j}",
                                     name=f"sh_t2_{j}")
                P1f_s = persist.tile([128, WFD], f32, tag=f"shP1_{j}",
                                     name=f"sh_P1_{j}")
                ux2_s = persist.tile([128, WFD], f16, tag=f"shux2_{j}",
                                     name=f"sh_ux2_{j}")
                ux3_s = persist.tile([128, WFD], f16, tag=f"shux3_{j}",
                                     name=f"sh_ux3_{j}")
                sh[j] = (t2f_s, P1f_s, ux2_s, ux3_s)
                ge = nc.gpsimd if j < 2 else nc.vector

                zw, aw = [], []
                for hx in range(2):
                    hs = slice(hx * FD, (hx + 1) * FD)
                    z2 = ps.tile([128, FD], f32, tag=zpair[0][hx], name=f"z2_{j}{hx}")
                    nc.tensor.matmul(z2[:], smt["wt2h"], t1h[:, hs], start=True,
                                     stop=False)
                    nc.tensor.matmul(z2[:], smt["wt2l"], t1h[:, hs], start=False,
                                     stop=True)
                    zw.append(z2)
                    A = ps.tile([128, FD], f32, tag=zpair[1][hx], name=f"A_{j}{hx}")
                    nc.tensor.matmul(A[:], smt["w2ah"], tp1h[:, hs], start=True,
                                     stop=False)
                    nc.tensor.matmul(A[:], smt["w2al"], tp1h[:, hs], start=False,
                                     stop=True)
                    aw.append(A)

                Bc = scr.tile([128, WFD], f16, tag="Bc", name=f"Bc_{j}")
                Cc = scr.tile([128, WFD], f16, tag="Cc", name=f"Cc_{j}")
                A2c = scr.tile([128, WFD], f16, tag="A2c", name=f"A2c_{j}")
                Acp = scr.tile([128, WFD], f16, tag="Acp", name=f"Acp_{j}")
                for hx in range(2):
                    hs = slice(hx * FD, (hx + 1) * FD)
                    nc.scalar.activation(t2f_s[:, hs], zw[hx][:], AF.Tanh,
                                         bias=smt["bt2b"])
                    nc.scalar.square(A2c[:, hs], aw[hx][:])
                    nc.scalar.copy(Acp[:, hs], aw[hx][:])
                    B = ps.tile([128, FD], f32, tag="pB", name=f"B_{j}{hx}")
                    nc.tensor.matmul(B[:], smt["w2b"], g2m[:, hs], start=True,
                                     stop=True)
                    nc.scalar.copy(Bc[:, hs], B[:])
                    C = ps.tile([128, FD], f32, tag="pC", name=f"C_{j}{hx}")
                    nc.tensor.matmul(C[:], smt["w2c"], g3m[:, hs], start=True,
                                     stop=True)
                    nc.scalar.copy(Cc[:, hs], C[:])

                t2c = scr.tile([128, WFD], f16, tag="t2c", name=f"t2c_{j}")
                nc.scalar.copy(t2c[:], t2f_s[:])
                s2 = scr.tile([128, WFD], f32, tag=f"s2_{j % 2}", name=f"s2_{j}")
                nc.scalar.square(s2[:], t2f_s[:])
                tp2 = scr.tile([128, WFD], f32, tag=f"tp2_{j % 2}", name=f"tp2_{j}")
                nc.vector.tensor_scalar(tp2[:], s2[:], -1.0, 1.0, ALU.mult, ALU.add)
                tp2c = scr.tile([128, WFD], f16, tag="tp2c", name=f"tp2c_{j}")
                nc.scalar.activation(tp2c[:], s2[:], AF.Copy, bias=1.0, scale=-1.0)
                for hx in range(2):
                    hs = slice(hx * FD, (hx + 1) * FD)
                    TT(nc.vector, P1f_s[:, hs], tp2[:, hs], aw[hx][:])

                T1 = scr.tile([128, WFD], f16, tag="T1", name=f"T1_{j}")
                TT(ge, T1[:], t2c[:], A2c[:])
                E = scr.tile([128, WFD], f16, tag="E", name=f"E_{j}")
                nc.vector.scalar_tensor_tensor(
                    E[:], T1[:], -2.0, Bc[:], ALU.mult, ALU.add)
                TT(nc.vector, ux2_s[:], tp2c[:], E[:])
                A3 = scr.tile([128, WFD], f16, tag="A3", name=f"A3_{j}")
                TT(ge, A3[:], A2c[:], Acp[:])
                G1 = scr.tile([128, WFD], f16, tag="G1", name=f"G1_{j}")
                nc.vector.scalar_tensor_tensor(
                    G1[:], tp2c[:], 2.0 / 3.0, A3[:], ALU.subtract, ALU.mult)
                G2 = scr.tile([128, WFD], f16, tag="G2", name=f"G2_{j}")
                TT(ge, G2[:], t2c[:], Acp[:])
                G3 = scr.tile([128, WFD], f16, tag="G3", name=f"G3_{j}")
                TT(ge, G3[:], G2[:], Bc[:])
                D = scr.tile([128, WFD], f16, tag="Dd", name=f"D_{j}")
                TT(nc.vector, D[:], G1[:], G3[:], ALU.add)
                # Cc holds -C_true (g3m sign-flip): H = -6*D - Cc = -6*D + C
                H = scr.tile([128, WFD], f16, tag="Hh", name=f"H_{j}")
                nc.vector.scalar_tensor_tensor(
                    H[:], D[:], -6.0, Cc[:], ALU.mult, ALU.subtract)
                TT(nc.vector, ux3_s[:], tp2c[:], H[:])

            # ---- b -> c -> outer-product stationaries S = c(x)p, c(x)q ----
            with tc.high_priority():
                b16 = smp.tile([128, 1], f16, name="b16")
                nc.scalar.copy(b16[:], b_ar[:])
                c0p = ps.tile([1, 128], f32, tag="pBC", name="c0p")
                nc.tensor.matmul(c0p[:], b16[:], smt["wt3h"], start=True, stop=False)
                nc.tensor.matmul(c0p[:], b16[:], smt["wt3l"], start=False, stop=True)
                c0sb = smp.tile([1, 128], f16, name="c0sb")
                nc.scalar.copy(c0sb[:], c0p[:])
                scpq_p = ps.tile([128, 128], f32, tag="pBC", name="scpq_p")
                nc.tensor.matmul(scpq_p[:], c0sb[:], smt["pq2"], start=True, stop=True)
                Scpq32 = smp.tile([128, 128], f32, name="Scpq32")
                nc.scalar.copy(Scpq32[:], scpq_p[:])
                Scpq16 = smp.tile([128, 128], f16, name="Scpq16")
                nc.scalar.copy(Scpq16[:], scpq_p[:])
            Sp32, Sq32 = Scpq32[:, 0:64], Scpq32[:, 64:128]
            Sp16, Sq16 = Scpq16[:, 0:64], Scpq16[:, 64:128]

            # ---- energy phase ----
            for e in range(NEN):
                t2f_s, P1f_s, ux2_s, ux3_s = sh[e]
                trio = [["zA", "zB", "aA"], ["aB", "pB", "pC"]][e % 2]
                dzt, dyt = ("pBC", "pMV") if e % 2 == 0 else ("pMV", "pBC")

                z1e = ps.tile([128, FD], f32, tag=trio[0], name=f"z1e_{e}")
                z1p = ps.tile([128, FD], f32, tag=trio[1], name=f"z1p_{e}")
                z1pp = ps.tile([128, FD], f32, tag=trio[2], name=f"z1pp_{e}")
                for hx in range(2):
                    hs = slice(hx * FD, (hx + 1) * FD)
                    rs = slice(hx * 64, (hx + 1) * 64)
                    nc.tensor.matmul(z1e[rs, :], Sp32, t2f_s[:, hs], start=True,
                                     stop=False)
                    nc.tensor.matmul(z1e[rs, :], Sq32, P1f_s[:, hs], start=False,
                                     stop=True)
                    nc.tensor.matmul(z1p[rs, :], Sp32, P1f_s[:, hs], start=True,
                                     stop=False)
                    nc.tensor.matmul(z1p[rs, :], Sq16, ux2_s[:, hs], start=False,
                                     stop=True)
                    nc.tensor.matmul(z1pp[rs, :], Sp16, ux2_s[:, hs], start=True,
                                     stop=False)
                    nc.tensor.matmul(z1pp[rs, :], Sq16, ux3_s[:, hs], start=False,
                                     stop=True)

                t1ef = scr.tile([128, FD], f32, tag="t1ef", name=f"t1ef_{e}")
                nc.scalar.activation(t1ef[:], z1e[:], AF.Tanh, bias=smt["be1b2"])
                t1eh = scr.tile([128, FD], f16, tag="t1eh", name=f"t1eh_{e}")
                nc.scalar.copy(t1eh[:], t1ef[:])
                z1psb = scr.tile([128, FD], f16, tag="z1psb", name=f"z1psb_{e}")
                nc.scalar.copy(z1psb[:], z1p[:])
                z1ppsb = scr.tile([128, FD], f16, tag="z1ppsb", name=f"z1ppsb_{e}")
                nc.scalar.copy(z1ppsb[:], z1pp[:])
                s1e = scr.tile([128, FD], f16, tag="s1e", name=f"s1e_{e}")
                nc.scalar.square(s1e[:], t1ef[:])
                m_ = scr.tile([128, FD], f16, tag="m_", name=f"m_{e}")
                nc.scalar.activation(m_[:], s1e[:], AF.Copy, bias=1.0, scale=-1.0)
                z1p2 = scr.tile([128, FD], f16, tag="z1p2", name=f"z1p2_{e}")
                TT(nc.gpsimd, z1p2[:], z1psb[:], z1psb[:])
                N1 = scr.tile([128, FD], f16, tag="N1", name=f"N1_{e}")
                TT(nc.vector, N1[:], t1eh[:], m_[:])
                a1p = scr.tile([128, FD], f16, tag="a1p", name=f"a1p_{e}")
                TT(nc.vector, a1p[:], m_[:], z1psb[:])
                N2 = scr.tile([128, FD], f16, tag="N2", name=f"N2_{e}")
                TT(nc.gpsimd, N2[:], N1[:], z1p2[:])
                N3 = scr.tile([128, FD], f16, tag="N3", name=f"N3_{e}")
                TT(nc.vector, N3[:], m_[:], z1ppsb[:])
                zin = scr.tile([128, FD], f16, tag="zin", name=f"zin_{e}")
                nc.vector.scalar_tensor_tensor(
                    zin[:], N2[:], -2.0, N3[:], ALU.mult, ALU.add)
                mpc = scr.tile([128, FD], f16, tag="mpc", name=f"mpc_{e}")
                TT(nc.vector, mpc[:], N1[:], z1psb[:])
                O1 = scr.tile([128, FD], f16, tag="O1", name=f"O1_{e}")
                nc.vector.scalar_tensor_tensor(
                    O1[:], s1e[:], 1.0 / 3.0, m_[:], ALU.subtract, ALU.mult)
                O2f = scr.tile([128, FD], f16, tag="O2f", name=f"O2f_{e}")
                TT(nc.gpsimd, O2f[:], O1[:], z1p2[:])
                O3f = scr.tile([128, FD], f16, tag="O3f", name=f"O3f_{e}")
                TT(nc.vector, O3f[:], N1[:], z1ppsb[:])
                O2m = scr.tile([128, FD], f16, tag="O2m", name=f"O2m_{e}")
                nc.vector.scalar_tensor_tensor(
                    O2m[:], O2f[:], 3.0, O3f[:], ALU.mult, ALU.subtract)

                z2e = ps.tile([128, FD], f32, tag=trio[0], name=f"z2e_{e}")
                nc.tensor.matmul(z2e[:], smt["e0"], t1eh[:], start=True, stop=True)
                z2ep = ps.tile([128, FD], f32, tag=trio[1], name=f"z2ep_{e}")
                nc.tensor.matmul(z2ep[:], smt["e0"], a1p[:], start=True, stop=True)
                z2epp = ps.tile([128, FD], f32, tag=trio[2], name=f"z2epp_{e}")
                nc.tensor.matmul(z2epp[:], smt["e0"], zin[:], start=True, stop=True)
                Dz = ps.tile([128, FD], f32, tag=dzt, name=f"Dz_{e}")
                nc.tensor.matmul(Dz[:], smt["eq"], m_[:], start=True, stop=True)
                DyN = ps.tile([128, FD], f32, tag=dyt, name=f"DyN_{e}")
                nc.tensor.matmul(DyN[:], smt["ep"], m_[:], start=True, stop=True)
                DzpN = ps.tile([128, FD], f32, tag=trio[0], name=f"DzpN_{e}")
                nc.tensor.matmul(DzpN[:], smt["eq"], mpc[:], start=True, stop=True)
                DypN = ps.tile([128, FD], f32, tag=trio[1], name=f"DypN_{e}")
                nc.tensor.matmul(DypN[:], smt["ep"], mpc[:], start=True, stop=True)
                Dzpp2 = ps.tile([128, FD], f32, tag=trio[2], name=f"Dzpp2_{e}")
                nc.tensor.matmul(Dzpp2[:], smt["eq"], O2m[:], start=True, stop=True)

                t2e = scr.tile([128, FD], f16, tag="t2e", name=f"t2e_{e}")
                nc.scalar.activation(t2e[:], z2e[:], AF.Tanh, bias=smt["be2b2"])
                s2e = scr.tile([128, FD], f16, tag="s2e", name=f"s2e_{e}")
                nc.scalar.square(s2e[:], t2e[:])
                w_ = scr.tile([128, FD], f16, tag="w_", name=f"w_{e}")
                nc.scalar.activation(w_[:], s2e[:], AF.Copy, bias=1.0, scale=-1.0)
                z2ep16 = scr.tile([128, FD], f16, tag="z2ep16", name=f"z2ep16_{e}")
                nc.scalar.copy(z2ep16[:], z2ep[:])
                z2ep2 = scr.tile([128, FD], f16, tag="z2ep2", name=f"z2ep2_{e}")
                TT(nc.gpsimd, z2ep2[:], z2ep16[:], z2ep16[:])
                Q1 = scr.tile([128, FD], f16, tag="Q1", name=f"Q1_{e}")
                TT(nc.vector, Q1[:], t2e[:], w_[:])
                wpc = scr.tile([128, FD], f16, tag="wpc", name=f"wpc_{e}")
                TT(nc.vector, wpc[:], Q1[:], z2ep16[:])
                R1 = scr.tile([128, FD], f16, tag="R1", name=f"R1_{e}")
                nc.vector.scalar_tensor_tensor(
                    R1[:], s2e[:], 1.0 / 3.0, w_[:], ALU.subtract, ALU.mult)
                R2f = scr.tile([128, FD], f16, tag="R2f", name=f"R2f_{e}")
                TT(nc.gpsimd, R2f[:], R1[:], z2ep2[:])
                R3f = scr.tile([128, FD], f16, tag="R3f", name=f"R3f_{e}")
                TT(nc.vector, R3f[:], Q1[:], z2epp[:])
                t1m = scr.tile([128, FD], f16, tag="t1m", name=f"t1m_{e}")
                nc.vector.scalar_tensor_tensor(
                    t1m[:], R2f[:], 3.0, R3f[:], ALU.mult, ALU.subtract)
                F1 = scr.tile([128, FD], f16, tag="F1", name=f"F1_{e}")
                TT(nc.vector, F1[:], t1m[:], Dz[:])
                DyNs = scr.tile([128, FD], f16, tag="DyNs", name=f"DyNs_{e}")
                nc.scalar.copy(DyNs[:], DyN[:])
                t2m = scr.tile([128, FD], f16, tag="t2m", name=f"t2m_{e}")
                nc.vector.scalar_tensor_tensor(
                    t2m[:], DzpN[:], 4.0, DyNs[:], ALU.mult, ALU.add)
                F2 = scr.tile([128, FD], f16, tag="F2", name=f"F2_{e}")
                TT(nc.gpsimd, F2[:], wpc[:], t2m[:])
                DypNs = scr.tile([128, FD], f16, tag="DypNs", name=f"DypNs_{e}")
                nc.scalar.copy(DypNs[:], DypN[:])
                t3m = scr.tile([128, FD], f16, tag="t3m", name=f"t3m_{e}")
                TT(nc.vector, t3m[:], Dzpp2[:], DypNs[:], ALU.add)
                F3 = scr.tile([128, FD], f16, tag="F3", name=f"F3_{e}")
                TT(nc.vector, F3[:], w_[:], t3m[:])

                vps = ps.tile([2, FD], f32, tag=trio[1], name=f"vps_{e}")
                nc.tensor.matmul(vps[:], smt["v6"][:, 0:2], F1[:], start=True,
                                 stop=False)
                nc.tensor.matmul(vps[:], smt["v6"][:, 2:4], F2[:], start=False,
                                 stop=False)
                nc.tensor.matmul(vps[:], smt["v6"][:, 4:6], F3[:], start=False,
                                 stop=True)
                ot = scr.tile([2, FD], f32, tag="ot", name=f"ot_{e}")
                nc.scalar.copy(ot[:], vps[:])
                nc.sync.dma_start(out_d.ap()[:, e * FD:(e + 1) * FD], ot[:])

    nc.compile()
    return nc


def _get_nc():
    if "nc" not in _CACHE:
        _CACHE["nc"] = _build()
    return _CACHE["nc"]


def kernel(**inputs):
    import concourse.bass_utils as bass_utils

    f = lambda k: np.asarray(inputs[k], np.float32)
    a, x, t = f("a"), f("x"), np.float32(inputs["t"])
    Wb, Wt1, bt1, Wt2, bt2 = f("Wb"), f("Wt1"), f("bt1"), f("Wt2"), f("bt2")
    Wt3, We1, be1, We2, be2, We3 = (
        f("Wt3"), f("We1"), f("be1"), f("We2"), f("be2"), f("We3"))

    h16 = lambda v: np.asarray(v, np.float32).astype(np.float16)
    def pair16(v):
        h = h16(v)
        return h, h16(np.asarray(v, np.float32) - h.astype(np.float32))

    w1 = Wt1[:, 0]
    c1b = (Wt1[:, 1] * t + bt1)[:, None]
    w1h, w1l = pair16(w1)
    w11 = np.stack([w1h, w1h, w1l, w1l])                       # [4,128]
    wt2t = np.ascontiguousarray(Wt2.T)
    wt2h, wt2l = pair16(wt2t)
    w2ah, w2al = pair16(wt2t * w1[:, None])
    w2b = h16(wt2t * (-2.0 * w1 ** 2)[:, None])
    w2c = h16(wt2t * (6.0 * w1 ** 3)[:, None])
    wt3h, wt3l = pair16(Wt3)

    p, q, v = We1[:, 0], We1[:, 1], We3[0]
    pq2 = np.zeros((1, 128), np.float16)
    pq2[0, 0:64] = h16(p)
    pq2[0, 64:128] = h16(q)

    blk = lambda M: np.block([[M, np.zeros_like(M)], [np.zeros_like(M), M]])
    We2T = We2.T
    e0 = h16(blk(We2T))
    eq = h16(blk(We2T * q[:, None]))
    ep = h16(blk(We2T * p[:, None]))
    v6 = np.zeros((128, 6), np.float16)
    for i in range(3):
        v6[0:64, 2 * i] = h16(2.0 * v)
        v6[64:128, 2 * i + 1] = h16(2.0 * v)
    sel4m = np.zeros((8, 4), np.float32)
    for j in range(4):
        sel4m[2 * j, j] = 1.0
        sel4m[2 * j + 1, j] = 1.0

    pkb = np.zeros((128, PKB_COLS), np.float16)
    for n_, arr in [("wt2h", wt2h), ("wt2l", wt2l), ("w2ah", w2ah),
                    ("w2al", w2al), ("w2b", w2b), ("w2c", w2c)]:
        pkb[:, _PKB[n_]:_PKB[n_] + 128] = arr
    pkc = np.zeros((128, PKC_COLS), np.float16)
    for n_, arr in [("wt3h", wt3h), ("wt3l", wt3l), ("e0", e0), ("eq", eq),
                    ("ep", ep)]:
        pkc[:, _PKC[n_]:_PKC[n_] + 128] = arr
    pkc[0:1, _PKC["pq2"]:_PKC["pq2"] + 128] = pq2
    pkc[:, _PKC["v6"]:_PKC["v6"] + 6] = v6
    pk32 = np.zeros((128, PK32_COLS), np.float32)
    pk32[:, 0] = c1b[:, 0]
    pk32[:, 1] = bt2
    pk32[:, 2] = np.concatenate([be1, be1])
    pk32[:, 3] = np.concatenate([be2, be2])
    pk32[0:8, 4:8] = sel4m

    smalls = {
        "w11": np.ascontiguousarray(w11),
        "pkb": np.ascontiguousarray(pkb),
        "pkc": np.ascontiguousarray(pkc),
        "pk32": np.ascontiguousarray(pk32),
    }

    in_maps = []
    for c in range(NCORES):
        blk_w = Wb[:, c * KSH:(c + 1) * KSH]                   # [128, 65536]
        tr = blk_w.T.reshape(NKT, 128, 128).transpose(1, 0, 2)  # [k1, kt, p]
        tr = tr.reshape(128, NCHUNK, KTC * 128).transpose(1, 0, 2)
        wsh = np.ascontiguousarray(h16(1024.0 * tr))           # [16,128,4096]
        ash = (a[c * KSH:(c + 1) * KSH] / 1024.0).reshape(NKT, 128).T  # [k1, kt]
        ah, al = pair16(ash)
        a2 = np.ascontiguousarray(np.stack([ah, al], axis=2))  # [128,512,2]
        xs = x[c * NPTS:(c + 1) * NPTS]
        xh, xl = pair16(xs)
        x4 = np.ascontiguousarray(np.stack([xh, xl, xh, xl]))  # [4,4096]
        im = {"w": wsh, "a2": a2, "x4": x4}
        im.update(smalls)
        in_maps.append(im)

    global _last_in_maps
    _last_in_maps = in_maps
    nc = _get_nc()
    res = bass_utils.run_bass_kernel_spmd(nc, in_maps, core_ids=list(range(NCORES)))
    outs = []
    for c in range(NCORES):
        o = res.results[c]["out"]          # [2, NPTS//2]
        outs.append(np.asarray(o).reshape(-1))
    return np.concatenate(outs).astype(np.float32)
